# revision 14
# baseline (speedup 1.0000x reference)
"""Trainium2 Bass kernel for nn_BasicRNNBlock (vanilla tanh RNN).

Reference semantics (fp32):
    xp = einsum("bti,hi->tbh", x, W_ih) + b_ih + b_hh      # input projection
    h_t = tanh(xp_t + h_{t-1} @ W_hh.T),  h_0 = 0          # T sequential steps
    out[b, t, :] = h_t[b]                                  # [B, T, H]

Shapes: B=64, T=512, I=H=1024.  Sharding: data-parallel over batch across
8 NeuronCores (8 batches/core, weights replicated).  All-fp16 matmul inputs,
fp32 PSUM accumulation.

Per-core device program (SPMD), v2:
  Phase 1 (prologue): the full input projection runs as one dense burst of
  512 N=512 matmuls (HAM-warm => ~2x faster streaming than interleaved); the
  resulting xp for all T steps stays resident in SBUF (8 MB) in transposed
  layout [kappa, s*4096 + c*512 + local*8 + b].
  Phase 2 (recurrence): per step, 64 fp16 LDW+MM pairs (W_hh 128x128 blocks
  stationary, hT [128, 8] moving) accumulate z^T into two PSUM tiles; an
  identity matmul injects xp_t (start=True).  MM order is staged by k-group
  so matmuls that consume the previous step's tanh-hi half issue last:
    ident_lo, ident_hi, [k0..3 x c0..7], [k4..7 x c0..3] -> tanh_lo,
    [k4..7 x c4..7] -> tanh_hi.
  This keeps the weight-load port saturated (~27 ns/block) with no
  step-boundary stalls.
"""
import numpy as np

B, T, I, H = 64, 512, 1024, 1024
N_CORES = 8
BS = B // N_CORES          # 8 batches per core
NCH = H // 128             # 8 chunks of 128 along H
WIN = 64                   # steps per projection slice (512 cols)
NSLICE = T // WIN          # 8 projection slices


def _build_program(steps=T):
    from concourse import bacc, mybir
    import concourse.tile as tile

    f16 = mybir.dt.float16
    f32 = mybir.dt.float32

    nc = bacc.Bacc(None, target_bir_lowering=False)

    wih = nc.declare_dram_parameter("wih", [128, 8192], f16, isOutput=False)
    whh = nc.declare_dram_parameter("whh", [128, 8192], f16, isOutput=False)
    xt = nc.declare_dram_parameter("xt", [128, 8 * 4096], f16, isOutput=False)
    ident = nc.declare_dram_parameter("ident", [128, 128], f16, isOutput=False)
    bias = nc.declare_dram_parameter("bias", [128, 8], f32, isOutput=False)
    y = nc.declare_dram_parameter("y", [steps, 128, 64], f16, isOutput=True)

    n_slices_used = (steps + WIN - 1) // WIN

    with tile.TileContext(nc) as tc:
        with (
            tc.tile_pool(name="const", bufs=1) as const_pool,
            tc.tile_pool(name="xslice", bufs=2) as xslice_pool,
            tc.tile_pool(name="xpall", bufs=1) as xpall_pool,
            tc.tile_pool(name="hst", bufs=3) as h_pool,
            tc.tile_pool(name="pp", bufs=2, space="PSUM") as proj_psum,
            tc.tile_pool(name="rp", bufs=3, space="PSUM") as rec_psum,
        ):
            wih_sb = const_pool.tile([128, 8192], f16)
            whh_sb = const_pool.tile([128, 8192], f16)
            ident_sb = const_pool.tile([128, 128], f16)
            bias_sb = const_pool.tile([128, 8], f32)
            nc.sync.dma_start(wih_sb[:], wih[:])
            nc.sync.dma_start(whh_sb[:], whh[:])
            nc.sync.dma_start(ident_sb[:], ident[:])
            nc.sync.dma_start(bias_sb[:], bias[:])

            eng_cycle = [nc.sync, nc.gpsimd]

            def load_xt_slice(s):
                """DMA xt k-chunks for slice s into a fresh [128, 4096] tile."""
                xsl = xslice_pool.tile([128, 8 * 512], f16, name="xsl", tag="xsl")
                for k in range(8):
                    eng_cycle[k % 2].dma_start(
                        xsl[:, k * 512:(k + 1) * 512],
                        xt[:, k * 4096 + s * 512: k * 4096 + (s + 1) * 512],
                    )
                return xsl

            # ------------- phase 1: full input projection (dense burst) ------
            # xp_all layout: [kappa, s*4096 + c*512 + local_t*8 + b]
            xp_all = xpall_pool.tile([128, n_slices_used * 4096], f16)

            xsl_next = load_xt_slice(0)
            for s in range(n_slices_used):
                xsl = xsl_next
                if s + 1 < n_slices_used:
                    xsl_next = load_xt_slice(s + 1)
                for c in range(NCH):
                    psum = proj_psum.tile([128, 512], f32, name="ppsum", tag="ppsum")
                    for k in range(8):
                        nc.tensor.matmul(
                            psum[:],
                            wih_sb[:, k * 1024 + c * 128: k * 1024 + (c + 1) * 128],
                            xsl[:, k * 512:(k + 1) * 512],
                            start=(k == 0), stop=(k == 7),
                        )
                    nc.vector.tensor_scalar_add(
                        xp_all[:, s * 4096 + c * 512: s * 4096 + (c + 1) * 512],
                        psum[:],
                        bias_sb[:, c:c + 1],
                    )

            # ------------- phase 2: recurrence -------------------------------
            h_cur = None
            for t in range(steps):
                s = t // WIN
                local = t - s * WIN
                xp3 = xp_all[:, s * 4096:(s + 1) * 4096].rearrange(
                    "p (c n) -> p c n", c=NCH)

                psum_lo = rec_psum.tile([128, 4, 8], f32, name="pslo", tag="pslo")
                psum_hi = rec_psum.tile([128, 4, 8], f32, name="pshi", tag="pshi")
                nc.tensor.matmul(
                    psum_lo[:], ident_sb[:], xp3[:, 0:4, local * 8:(local + 1) * 8],
                    start=True, stop=(t == 0), skip_group_check=True)
                nc.tensor.matmul(
                    psum_hi[:], ident_sb[:], xp3[:, 4:8, local * 8:(local + 1) * 8],
                    start=True, stop=(t == 0), skip_group_check=True)

                def wblock(c, k, last):
                    pt = psum_lo if c < 4 else psum_hi
                    nc.tensor.matmul(
                        pt[:, c % 4, :],
                        whh_sb[:, k * 1024 + c * 128: k * 1024 + (c + 1) * 128],
                        h_cur[:, k * 8:(k + 1) * 8],
                        start=False, stop=last,
                        skip_group_check=True,
                    )

                h_new = h_pool.tile([128, 64], f16)
                if t > 0:
                    # needs prev tanh_lo only (h chunks 0..3)
                    for k in range(4):
                        for c in range(8):
                            wblock(c, k, False)
                    # needs prev tanh_hi (h chunks 4..7); finish psum_lo first
                    for k in range(4, 8):
                        for c in range(4):
                            wblock(c, k, (k == 7 and c == 3))
                nc.scalar.activation(
                    h_new[:, 0:32],
                    psum_lo[:].rearrange("p c n -> p (c n)"),
                    mybir.ActivationFunctionType.Tanh,
                )
                if t > 0:
                    for k in range(4, 8):
                        for c in range(4, 8):
                            wblock(c, k, (k == 7 and c == 7))
                nc.scalar.activation(
                    h_new[:, 32:64],
                    psum_hi[:].rearrange("p c n -> p (c n)"),
                    mybir.ActivationFunctionType.Tanh,
                )
                nc.sync.dma_start(y[t], h_new[:])
                h_cur = h_new

    nc.compile()
    return nc


def _build_program_raw(steps=T):
    """Raw-bass build (no TileContext): the tile framework increments a
    per-engine progress semaphore on EVERY instruction, and those semaphore
    writes serialize at ~34 ns each — slower than the ~27 ns LDW+MM issue
    rate, making the semaphore unit the bottleneck (measured: step period
    2255 ns == 66 MMs x 34.2 ns).  Raw bass places semaphores only on the
    real dependency edges (2 psum stops + 2 tanh + DMAs per step).
    """
    from concourse import bacc, mybir
    import concourse.bass as bass

    f16 = mybir.dt.float16
    f32 = mybir.dt.float32
    Tanh = mybir.ActivationFunctionType.Tanh

    nc = bacc.Bacc(None, target_bir_lowering=False)

    wih = nc.declare_dram_parameter("wih", [128, 8192], f16, isOutput=False)
    whh = nc.declare_dram_parameter("whh", [128, 8192], f16, isOutput=False)
    xt = nc.declare_dram_parameter("xt", [128, 8 * 4096], f16, isOutput=False)
    ident = nc.declare_dram_parameter("ident", [128, 128], f16, isOutput=False)
    bias = nc.declare_dram_parameter("bias", [128, 8], f32, isOutput=False)
    y = nc.declare_dram_parameter("y", [steps, 128, 64], f16, isOutput=True)

    n_slices = (steps + WIN - 1) // WIN

    wih_sb = nc.alloc_sbuf_tensor("wih_sb", [128, 8192], f16)
    whh_sb = nc.alloc_sbuf_tensor("whh_sb", [128, 8192], f16)
    ident_sb = nc.alloc_sbuf_tensor("ident_sb", [128, 128], f16)
    bias_sb = nc.alloc_sbuf_tensor("bias_sb", [128, 8], f32)
    xp_all = nc.alloc_sbuf_tensor("xp_all", [128, n_slices * 4096], f16)
    xsl = [nc.alloc_sbuf_tensor(f"xsl{i}", [128, 4096], f16) for i in range(2)]
    hbuf = [nc.alloc_sbuf_tensor(f"hbuf{i}", [128, 64], f16) for i in range(4)]

    ystage = nc.alloc_sbuf_tensor("ystage", [128, 2 * 64], f16)

    ppsum = [nc.alloc_psum_tensor(f"ppsum{i}", [128, 512], f32) for i in range(2)]
    pslo = [nc.alloc_psum_tensor(f"pslo{i}", [128, 4, 8], f32) for i in range(3)]
    pshi = [nc.alloc_psum_tensor(f"pshi{i}", [128, 4, 8], f32) for i in range(3)]

    sem_const = nc.alloc_semaphore("sem_const")
    sem_xsl0 = nc.alloc_semaphore("sem_xsl0")
    sem_xsl1 = nc.alloc_semaphore("sem_xsl1")
    sem_proj = nc.alloc_semaphore("sem_proj")
    sem_xp = nc.alloc_semaphore("sem_xp")
    sem_pslo = nc.alloc_semaphore("sem_pslo")
    sem_pshi = nc.alloc_semaphore("sem_pshi")
    sem_hlo = nc.alloc_semaphore("sem_hlo")
    sem_hhi = nc.alloc_semaphore("sem_hhi")
    sem_stg = nc.alloc_semaphore("sem_stg")
    sem_y = nc.alloc_semaphore("sem_y")
    sem_yg = nc.alloc_semaphore("sem_yg")

    HB = 4   # hbuf depth

    with nc.Block() as block:

        @block.sync
        def _(sync):
            sync.dma_start(wih_sb[:], wih[:]).then_inc(sem_const, 16)
            sync.dma_start(whh_sb[:], whh[:]).then_inc(sem_const, 16)
            sync.dma_start(ident_sb[:], ident[:]).then_inc(sem_const, 16)
            sync.dma_start(bias_sb[:], bias[:]).then_inc(sem_const, 16)
            for s in range(n_slices):
                if s >= 2:
                    # xsl[s%2] consumed once proj of slice s-2 fully issued
                    sync.wait_ge(sem_proj, 8 * (s - 1))
                for k in (0, 2, 4, 6):
                    sync.dma_start(
                        xsl[s % 2][:, k * 512:(k + 1) * 512],
                        xt[:, k * 4096 + s * 512: k * 4096 + (s + 1) * 512],
                    ).then_inc(sem_xsl0, 16)
            for t in range(steps):
                # output DMA reads the staging copy, not hbuf, so it never
                # contends with the PE's h-operand stream on SBUF reads
                slot = t % 2
                sync.wait_ge(sem_stg, t + 1)
                sync.dma_start(
                    y[t], ystage[:, slot * 64:(slot + 1) * 64]
                ).then_inc(sem_y, 16)

        @block.gpsimd
        def _(gpsimd):
            for s in range(n_slices):
                if s >= 2:
                    gpsimd.wait_ge(sem_proj, 8 * (s - 1))
                for k in (1, 3, 5, 7):
                    gpsimd.dma_start(
                        xsl[s % 2][:, k * 512:(k + 1) * 512],
                        xt[:, k * 4096 + s * 512: k * 4096 + (s + 1) * 512],
                    ).then_inc(sem_xsl1, 16)

        @block.tensor
        def _(tensor):
            tensor.wait_ge(sem_const, 64)
            # ---- phase 1: input projection, dense warm burst ----
            for s in range(n_slices):
                tensor.wait_ge(sem_xsl0, 64 * (s + 1))
                tensor.wait_ge(sem_xsl1, 64 * (s + 1))
                for c in range(NCH):
                    idx = 8 * s + c
                    if idx >= 2:
                        tensor.wait_ge(sem_xp, idx - 1)  # ppsum ping-pong WAR
                    for k in range(8):
                        mm = tensor.matmul(
                            ppsum[idx % 2][:],
                            wih_sb[:, k * 1024 + c * 128: k * 1024 + (c + 1) * 128],
                            xsl[s % 2][:, k * 512:(k + 1) * 512],
                            start=(k == 0), stop=(k == 7),
                        )
                        if k == 7:
                            mm.then_inc(sem_proj, 1)

            # ---- phase 2: recurrence ----
            for t in range(steps):
                s = t // WIN
                local = t - s * WIN
                if local == 0:
                    tensor.wait_ge(sem_xp, 8 * (s + 1))
                xp3 = xp_all[:, s * 4096:(s + 1) * 4096].rearrange(
                    "p (c n) -> p c n", c=NCH)
                lo = pslo[t % 3]
                hi = pshi[t % 3]
                mm = tensor.matmul(
                    lo[:], ident_sb[:], xp3[:, 0:4, local * 8:(local + 1) * 8],
                    start=True, stop=(t == 0), skip_group_check=True)
                if t == 0:
                    mm.then_inc(sem_pslo, 1)
                mm = tensor.matmul(
                    hi[:], ident_sb[:], xp3[:, 4:8, local * 8:(local + 1) * 8],
                    start=True, stop=(t == 0), skip_group_check=True)
                if t == 0:
                    mm.then_inc(sem_pshi, 1)

                if t == 0:
                    continue
                h_prev = hbuf[(t - 1) % HB]

                def wblock(c, k, last, sem=None):
                    pt = lo if c < 4 else hi
                    mm = tensor.matmul(
                        pt[:, c % 4, :],
                        whh_sb[:, k * 1024 + c * 128: k * 1024 + (c + 1) * 128],
                        h_prev[:, k * 8:(k + 1) * 8],
                        start=False, stop=last,
                        skip_group_check=True,
                    )
                    if sem is not None:
                        mm.then_inc(sem, 1)

                # needs h chunks 0..3 only (prev tanh_lo)
                tensor.wait_ge(sem_hlo, t)
                for k in range(4):
                    for c in range(4):
                        wblock(c, k, False)
                for k in range(2):
                    for c in range(4, 8):
                        wblock(c, k, False)
                # needs h chunks 4..7 (prev tanh_hi)
                tensor.wait_ge(sem_hhi, t)
                for k in range(4, 8):
                    for c in range(4):
                        wblock(c, k, (k == 7 and c == 3),
                               sem_pslo if (k == 7 and c == 3) else None)
                for k in range(2, 4):
                    for c in range(4, 8):
                        wblock(c, k, False)
                for k in range(4, 8):
                    for c in range(4, 8):
                        wblock(c, k, (k == 7 and c == 7),
                               sem_pshi if (k == 7 and c == 7) else None)

        @block.scalar
        def _(scalar):
            for t in range(steps):
                scalar.wait_ge(sem_pslo, t + 1)
                if t >= HB:
                    scalar.wait_ge(sem_stg, t - HB + 1)  # hbuf WAR vs stage copy
                scalar.activation(
                    hbuf[t % HB][:, 0:32],
                    pslo[t % 3][:].rearrange("p c n -> p (c n)"),
                    Tanh,
                ).then_inc(sem_hlo, 1)
                scalar.wait_ge(sem_pshi, t + 1)
                scalar.activation(
                    hbuf[t % HB][:, 32:64],
                    pshi[t % 3][:].rearrange("p c n -> p (c n)"),
                    Tanh,
                ).then_inc(sem_hhi, 1)

        @block.vector
        def _(vector):
            vector.wait_ge(sem_const, 64)
            for s in range(n_slices):
                for c in range(NCH):
                    idx = 8 * s + c
                    vector.wait_ge(sem_proj, idx + 1)
                    vector.tensor_scalar_add(
                        xp_all[:, s * 4096 + c * 512: s * 4096 + (c + 1) * 512],
                        ppsum[idx % 2][:],
                        bias_sb[:, c:c + 1],
                    ).then_inc(sem_xp, 1)
            for t in range(steps):
                slot = t % 2
                vector.wait_ge(sem_hhi, t + 1)
                if t >= 2:
                    vector.wait_ge(sem_y, 16 * (t - 1))   # stage slot WAR
                vector.tensor_copy(
                    ystage[:, slot * 64:(slot + 1) * 64], hbuf[t % HB][:]
                ).then_inc(sem_stg, 1)

    nc.compile()
    return nc


def _build_program_loop(steps=T):
    """Raw bass + hardware loop.

    The straight-line raw program stalls ~450-870 ns at every 16 KB
    instruction-page boundary (pc % 256 == 0): the tensor sequencer consumes
    64 B instructions at ~4.8 GB/s and the pager does not prefetch ahead.
    Fix: run the recurrence as a per-engine hardware loop whose one-step body
    (~150 instructions, <16 KB) stays resident in IRAM.  Buffer rotation
    (h/psum double-buffers, xp/y offsets, semaphore thresholds) is done with
    engine registers; all weight addresses stay constant (LDWEIGHTS cannot
    take register offsets, matmul rhs/out can).

    xp layout here is c-major: col = c*4096 + t*8 + b, so the ident-matmul rhs
    offset is linear in t (offset = t*8, hi at +16384).
    """
    from concourse import bacc, mybir
    import concourse.bass as bass

    f16 = mybir.dt.float16
    f32 = mybir.dt.float32
    Tanh = mybir.ActivationFunctionType.Tanh

    nc = bacc.Bacc(None, target_bir_lowering=False)

    wih = nc.declare_dram_parameter("wih", [128, 8192], f16, isOutput=False)
    whh = nc.declare_dram_parameter("whh", [128, 8192], f16, isOutput=False)
    xt = nc.declare_dram_parameter("xt", [128, 8 * 4096], f16, isOutput=False)
    ident = nc.declare_dram_parameter("ident", [128, 128], f16, isOutput=False)
    bias = nc.declare_dram_parameter("bias", [128, 8], f32, isOutput=False)
    y = nc.declare_dram_parameter("y", [steps, 128, 64], f16, isOutput=True)

    n_slices = (steps + WIN - 1) // WIN
    HEAD = min(8, steps)
    NLOOP = steps - HEAD

    wih_sb = nc.alloc_sbuf_tensor("wih_sb", [128, 8192], f16)
    whh_sb = nc.alloc_sbuf_tensor("whh_sb", [128, 8192], f16)
    ident_sb = nc.alloc_sbuf_tensor("ident_sb", [128, 128], f16)
    bias_sb = nc.alloc_sbuf_tensor("bias_sb", [128, 8], f32)
    xp_all = nc.alloc_sbuf_tensor("xp_all", [128, n_slices * 4096], f16)
    xsl = [nc.alloc_sbuf_tensor(f"xsl{i}", [128, 4096], f16) for i in range(2)]
    hball = nc.alloc_sbuf_tensor("hball", [128, 128], f16)   # 2 slots x 64
    ystage = nc.alloc_sbuf_tensor("ystage", [128, 64], f16)

    ppsum = [nc.alloc_psum_tensor(f"ppsum{i}", [128, 512], f32) for i in range(2)]
    pslo2 = nc.alloc_psum_tensor("pslo2", [128, 1024], f32)  # 2 slots x bank
    pshi2 = nc.alloc_psum_tensor("pshi2", [128, 1024], f32)

    sem_const = nc.alloc_semaphore("sem_const")
    sem_xsl0 = nc.alloc_semaphore("sem_xsl0")
    sem_xsl1 = nc.alloc_semaphore("sem_xsl1")
    sem_proj = nc.alloc_semaphore("sem_proj")
    sem_xp = nc.alloc_semaphore("sem_xp")
    sem_pslo = nc.alloc_semaphore("sem_pslo")
    sem_pshi = nc.alloc_semaphore("sem_pshi")
    sem_hlo = nc.alloc_semaphore("sem_hlo")
    sem_hhi = nc.alloc_semaphore("sem_hhi")
    sem_stg = nc.alloc_semaphore("sem_stg")
    sem_y = nc.alloc_semaphore("sem_y")

    # AP patterns (probed shapes; offsets in elements)
    P_XP3 = [[n_slices * 4096, 128], [4096, 4], [1, 8]]    # ident rhs view
    P_PS8 = [[1024, 128], [1, 8]]                           # one c-region
    P_PS32 = [[1024, 128], [1, 32]]                         # ident out / ACT src
    P_H8 = [[128, 128], [1, 8]]                             # one h chunk
    P_H32 = [[128, 128], [1, 32]]                           # ACT dst half
    P_H64 = [[128, 128], [1, 64]]                           # stage copy src
    P_Y = [[64, 128], [1, 64]]                              # y[t]

    def ap(tensor, off, pat):
        return bass.AP(tensor, off, pat)

    with nc.Block() as block:

        @block.sync
        def _(sync):
            sync.dma_start(wih_sb[:], wih[:]).then_inc(sem_const, 16)
            sync.dma_start(whh_sb[:], whh[:]).then_inc(sem_const, 16)
            sync.dma_start(ident_sb[:], ident[:]).then_inc(sem_const, 16)
            sync.dma_start(bias_sb[:], bias[:]).then_inc(sem_const, 16)
            for s in range(n_slices):
                if s >= 2:
                    sync.wait_ge(sem_proj, 8 * (s - 1))
                for k in (0, 2, 4, 6):
                    sync.dma_start(
                        xsl[s % 2][:, k * 512:(k + 1) * 512],
                        xt[:, k * 4096 + s * 512: k * 4096 + (s + 1) * 512],
                    ).then_inc(sem_xsl0, 16)
            for t in range(HEAD):
                sync.wait_ge(sem_stg, t + 1)
                sync.dma_start(y[t], ystage[:]).then_inc(sem_y, 16)
            if NLOOP > 0:
                y_thr = sync.alloc_register("y_thr")
                y_off = sync.alloc_register("y_off")
                sync.reg_mov(y_thr, HEAD)
                sync.reg_mov(y_off, HEAD * 8192)
                with sync.Fori(0, NLOOP):
                    sync.reg_add(y_thr, y_thr, 1)
                    sync.wait_ge(sem_stg, y_thr)
                    sync.dma_start(
                        ap(y, y_off, P_Y), ystage[:]
                    ).then_inc(sem_y, 16)
                    sync.reg_add(y_off, y_off, 8192)

        @block.gpsimd
        def _(gpsimd):
            for s in range(n_slices):
                if s >= 2:
                    gpsimd.wait_ge(sem_proj, 8 * (s - 1))
                for k in (1, 3, 5, 7):
                    gpsimd.dma_start(
                        xsl[s % 2][:, k * 512:(k + 1) * 512],
                        xt[:, k * 4096 + s * 512: k * 4096 + (s + 1) * 512],
                    ).then_inc(sem_xsl1, 16)

        @block.tensor
        def _(tensor):
            tensor.wait_ge(sem_const, 64)
            # ---- phase 1: input projection ----
            for s in range(n_slices):
                tensor.wait_ge(sem_xsl0, 64 * (s + 1))
                tensor.wait_ge(sem_xsl1, 64 * (s + 1))
                for c in range(NCH):
                    idx = 8 * s + c
                    if idx >= 2:
                        tensor.wait_ge(sem_xp, idx - 1)
                    for k in range(8):
                        mm = tensor.matmul(
                            ppsum[idx % 2][:],
                            wih_sb[:, k * 1024 + c * 128: k * 1024 + (c + 1) * 128],
                            xsl[s % 2][:, k * 512:(k + 1) * 512],
                            start=(k == 0), stop=(k == 7),
                        )
                        if k == 7:
                            mm.then_inc(sem_proj, 1)

            # ---- phase 2 helpers ----
            def w_lhsT(c, k):
                return whh_sb[:, k * 1024 + c * 128: k * 1024 + (c + 1) * 128]

            def emit_step(t, pslo_off, pshi_off, xp_off, xph_off, h_off,
                          ps_c, hk, wait_lo, wait_hi):
                """pslo_off/...: ScalarInput offsets; ps_c[c]: per-c psum
                offsets (c 0..7); hk[k]: h chunk offsets; wait_*: callables."""
                mm = tensor.matmul(
                    ap(pslo2, pslo_off, P_PS32), ident_sb[:],
                    ap(xp_all, xp_off, P_XP3),
                    start=True, stop=(t == 0), skip_group_check=True)
                if t == 0:
                    mm.then_inc(sem_pslo, 1)
                mm = tensor.matmul(
                    ap(pshi2, pshi_off, P_PS32), ident_sb[:],
                    ap(xp_all, xph_off, P_XP3),
                    start=True, stop=(t == 0), skip_group_check=True)
                if t == 0:
                    mm.then_inc(sem_pshi, 1)
                if t == 0:
                    return

                def wblock(c, k, last, sem=None):
                    pt = pslo2 if c < 4 else pshi2
                    mm = tensor.matmul(
                        ap(pt, ps_c[c], P_PS8),
                        w_lhsT(c, k),
                        ap(hball, hk[k], P_H8),
                        start=False, stop=last, skip_group_check=True)
                    if sem is not None:
                        mm.then_inc(sem, 1)

                wait_lo()
                for k in range(4):
                    for c in range(4):
                        wblock(c, k, False)
                for k in range(2):
                    for c in range(4, 8):
                        wblock(c, k, False)
                wait_hi()
                for k in range(4, 8):
                    for c in range(4):
                        wblock(c, k, (k == 7 and c == 3),
                               sem_pslo if (k == 7 and c == 3) else None)
                for k in range(2, 4):
                    for c in range(4, 8):
                        wblock(c, k, False)
                for k in range(4, 8):
                    for c in range(4, 8):
                        wblock(c, k, (k == 7 and c == 7),
                               sem_pshi if (k == 7 and c == 7) else None)

            # ---- head steps (literal addressing) ----
            for t in range(HEAD):
                s = t // WIN
                if t % WIN == 0:
                    tensor.wait_ge(sem_xp, 8 * (s + 1))
                slot = (t % 2) * 512
                hoff = ((t - 1) % 2) * 64
                emit_step(
                    t, slot, slot, t * 8, 16384 + t * 8, hoff,
                    [slot + (c % 4) * 8 for c in range(8)],
                    [hoff + k * 8 for k in range(8)],
                    (lambda tt=t: tensor.wait_ge(sem_hlo, tt)),
                    (lambda tt=t: tensor.wait_ge(sem_hhi, tt)),
                )

            # ---- loop steps (register addressing) ----
            if NLOOP > 0:
                # xp is fully materialized before t=8 only if HEAD>=... the
                # loop body cannot wait per-slice; wait for ALL slices now.
                tensor.wait_ge(sem_xp, 8 * n_slices)
                r_pslo = tensor.alloc_register("r_pslo")
                r_pshi = tensor.alloc_register("r_pshi")
                r_psc = [tensor.alloc_register(f"r_psc{j}") for j in range(1, 4)]
                r_phc = [tensor.alloc_register(f"r_phc{j}") for j in range(1, 4)]
                r_h = tensor.alloc_register("r_h")
                r_hk = [tensor.alloc_register(f"r_hk{k}") for k in range(1, 8)]
                r_xp = tensor.alloc_register("r_xp")
                r_xph = tensor.alloc_register("r_xph")
                r_tlo = tensor.alloc_register("r_tlo")
                r_thi = tensor.alloc_register("r_thi")
                # inits: flips happen at body start, so store the PREVIOUS value
                tensor.reg_mov(r_pslo, ((HEAD - 1) % 2) * 512)
                tensor.reg_mov(r_pshi, ((HEAD - 1) % 2) * 512)
                for j in range(1, 4):
                    tensor.reg_mov(r_psc[j - 1], ((HEAD - 1) % 2) * 512 + j * 8)
                    tensor.reg_mov(r_phc[j - 1], ((HEAD - 1) % 2) * 512 + j * 8)
                tensor.reg_mov(r_h, (HEAD % 2) * 64)       # flip -> (HEAD-1)%2
                for k in range(1, 8):
                    tensor.reg_mov(r_hk[k - 1], (HEAD % 2) * 64 + k * 8)
                tensor.reg_mov(r_xp, HEAD * 8)
                tensor.reg_mov(r_xph, 16384 + HEAD * 8)
                tensor.reg_mov(r_tlo, HEAD - 1)
                tensor.reg_mov(r_thi, HEAD - 1)

                with tensor.Fori(0, NLOOP):
                    tensor.reg_sub(r_pslo, 512, r_pslo)
                    tensor.reg_sub(r_pshi, 512, r_pshi)
                    for j in range(1, 4):
                        tensor.reg_sub(r_psc[j - 1], 512 + 16 * j, r_psc[j - 1])
                        tensor.reg_sub(r_phc[j - 1], 512 + 16 * j, r_phc[j - 1])
                    tensor.reg_sub(r_h, 64, r_h)
                    for k in range(1, 8):
                        tensor.reg_sub(r_hk[k - 1], 64 + 16 * k, r_hk[k - 1])
                    tensor.reg_add(r_tlo, r_tlo, 1)
                    tensor.reg_add(r_thi, r_thi, 1)
                    ps_c = ([r_pslo] + r_psc + [r_pshi] + r_phc)
                    hk = [r_h] + r_hk
                    emit_step(
                        -1, r_pslo, r_pshi, r_xp, r_xph, r_h,
                        ps_c, hk,
                        (lambda: tensor.wait_ge(sem_hlo, r_tlo)),
                        (lambda: tensor.wait_ge(sem_hhi, r_thi)),
                    )
                    tensor.reg_add(r_xp, r_xp, 8)
                    tensor.reg_add(r_xph, r_xph, 8)

        @block.scalar
        def _(scalar):
            def act_pair(pslo_off, pshi_off, hdst_lo, hdst_hi,
                         w_lo, w_stg, w_hi):
                w_lo()
                w_stg()
                scalar.activation(
                    ap(hball, hdst_lo, P_H32),
                    ap(pslo2, pslo_off, P_PS32), Tanh,
                ).then_inc(sem_hlo, 1)
                w_hi()
                scalar.activation(
                    ap(hball, hdst_hi, P_H32),
                    ap(pshi2, pshi_off, P_PS32), Tanh,
                ).then_inc(sem_hhi, 1)

            for t in range(HEAD):
                slot = (t % 2) * 512
                hd = (t % 2) * 64
                act_pair(
                    slot, slot, hd, hd + 32,
                    (lambda tt=t: scalar.wait_ge(sem_pslo, tt + 1)),
                    (lambda tt=t: scalar.wait_ge(sem_stg, tt - 1) if tt >= 2 else None),
                    (lambda tt=t: scalar.wait_ge(sem_pshi, tt + 1)),
                )
            if NLOOP > 0:
                s_ps = scalar.alloc_register("s_ps")
                s_ph = scalar.alloc_register("s_ph")
                s_hd = scalar.alloc_register("s_hd")
                s_hd32 = scalar.alloc_register("s_hd32")
                s_tlo = scalar.alloc_register("s_tlo")
                s_tst = scalar.alloc_register("s_tst")
                s_thi = scalar.alloc_register("s_thi")
                scalar.reg_mov(s_ps, ((HEAD - 1) % 2) * 512)
                scalar.reg_mov(s_ph, ((HEAD - 1) % 2) * 512)
                scalar.reg_mov(s_hd, ((HEAD - 1) % 2) * 64)
                scalar.reg_mov(s_tlo, HEAD)
                scalar.reg_mov(s_tst, HEAD - 2)
                scalar.reg_mov(s_thi, HEAD)
                with scalar.Fori(0, NLOOP):
                    scalar.reg_sub(s_ps, 512, s_ps)
                    scalar.reg_sub(s_ph, 512, s_ph)
                    scalar.reg_sub(s_hd, 64, s_hd)
                    scalar.reg_add(s_hd32, s_hd, 32)
                    scalar.reg_add(s_tlo, s_tlo, 1)
                    scalar.reg_add(s_tst, s_tst, 1)
                    scalar.reg_add(s_thi, s_thi, 1)
                    act_pair(
                        s_ps, s_ph, s_hd, s_hd32,
                        (lambda: scalar.wait_ge(sem_pslo, s_tlo)),
                        (lambda: scalar.wait_ge(sem_stg, s_tst)),
                        (lambda: scalar.wait_ge(sem_pshi, s_thi)),
                    )

        @block.vector
        def _(vector):
            vector.wait_ge(sem_const, 64)
            for s in range(n_slices):
                for c in range(NCH):
                    idx = 8 * s + c
                    vector.wait_ge(sem_proj, idx + 1)
                    vector.tensor_scalar_add(
                        xp_all[:, c * 4096 + s * 512: c * 4096 + (s + 1) * 512],
                        ppsum[idx % 2][:],
                        bias_sb[:, c:c + 1],
                    ).then_inc(sem_xp, 1)
            for t in range(HEAD):
                vector.wait_ge(sem_hhi, t + 1)
                if t >= 1:
                    vector.wait_ge(sem_y, 16 * t)
                vector.tensor_copy(
                    ystage[:], ap(hball, (t % 2) * 64, P_H64)
                ).then_inc(sem_stg, 1)
            if NLOOP > 0:
                v_thr = vector.alloc_register("v_thr")
                v_y = vector.alloc_register("v_y")
                v_hs = vector.alloc_register("v_hs")
                vector.reg_mov(v_thr, HEAD)
                vector.reg_mov(v_y, 16 * (HEAD - 1))
                vector.reg_mov(v_hs, ((HEAD - 1) % 2) * 64)
                with vector.Fori(0, NLOOP):
                    vector.reg_add(v_thr, v_thr, 1)
                    vector.wait_ge(sem_hhi, v_thr)
                    vector.reg_add(v_y, v_y, 16)
                    vector.wait_ge(sem_y, v_y)
                    vector.reg_sub(v_hs, 64, v_hs)
                    vector.tensor_copy(
                        ystage[:], ap(hball, v_hs, P_H64)
                    ).then_inc(sem_stg, 1)

    nc.compile()
    return nc


_PROGRAM_CACHE = {}
BUILD_KW = {"loop": True}


def _get_program(steps=T):
    key = (steps, tuple(sorted(BUILD_KW.items())))
    if key not in _PROGRAM_CACHE:
        kw = dict(BUILD_KW)
        if kw.pop("loop", False):
            builder = _build_program_loop
        elif kw.pop("raw", False):
            builder = _build_program_raw
        else:
            builder = _build_program
        _PROGRAM_CACHE[key] = builder(steps, **kw)
    return _PROGRAM_CACHE[key]


def _prep_shared(W_ih, W_hh, b_ih, b_hh):
    # lhsT layout [kappa, k*1024 + j] = W[j, k*128+kappa]
    def to_lhsT(W):
        return np.ascontiguousarray(
            W.T.reshape(8, 128, 1024).transpose(1, 0, 2).reshape(128, 8192)
        )

    wih_np = to_lhsT(np.asarray(W_ih)).astype(np.float16)
    whh_np = to_lhsT(np.asarray(W_hh)).astype(np.float16)
    bias_np = np.ascontiguousarray(
        (np.asarray(b_ih) + np.asarray(b_hh)).astype(np.float32).reshape(8, 128).T
    )
    ident_np = np.eye(128, dtype=np.float16)
    return wih_np, whh_np, bias_np, ident_np


TRACE = False
LAST_RESULT = [None]


def kernel(x, W_ih, W_hh, b_ih, b_hh, _steps=T):
    from concourse.bass_utils import run_bass_kernel_spmd

    x = np.asarray(x)
    steps = _steps
    nc = _get_program(steps)
    wih_np, whh_np, bias_np, ident_np = _prep_shared(W_ih, W_hh, b_ih, b_hh)

    in_maps = []
    for core in range(N_CORES):
        xs = x[core * BS:(core + 1) * BS]          # [8, T, I]
        # xt[kappa, k*4096 + t*8 + b] = x[b, t, k*128+kappa]
        xt_np = np.ascontiguousarray(
            xs.transpose(2, 1, 0)                   # [I, T, B]
            .reshape(8, 128, T * BS)                # [k, kappa, t*8+b]
            .transpose(1, 0, 2)                     # [kappa, k, t*8+b]
            .reshape(128, 8 * 4096)
        ).astype(np.float16)
        in_maps.append({
            "wih": wih_np, "whh": whh_np, "xt": xt_np,
            "ident": ident_np, "bias": bias_np,
        })

    res = run_bass_kernel_spmd(nc, in_maps, list(range(N_CORES)), trace=TRACE)
    LAST_RESULT[0] = res

    out = np.empty((B, T, H), dtype=np.float32)
    for core in range(N_CORES):
        yv = res.results[core]["y"]                 # [steps, 128, 64] fp16
        hb = (
            yv.reshape(steps, 128, 8, 8)
            .transpose(3, 0, 2, 1)                  # [b, t, c, kappa]
            .reshape(BS, steps, H)
            .astype(np.float32)
        )
        out[core * BS:(core + 1) * BS, :steps] = hb
    return out


# revision 21
# speedup vs baseline: 8.7591x; 8.7591x over previous
"""Trainium2 Bass kernel for nn_BasicRNNBlock (vanilla tanh RNN).

Reference semantics (fp32):
    xp = einsum("bti,hi->tbh", x, W_ih) + b_ih + b_hh      # input projection
    h_t = tanh(xp_t + h_{t-1} @ W_hh.T),  h_0 = 0          # T sequential steps
    out[b, t, :] = h_t[b]                                  # [B, T, H]

Shapes: B=64, T=512, I=H=1024.  Sharding: data-parallel over batch across
8 NeuronCores (8 batches/core, weights replicated).  All-fp16 matmul inputs,
fp32 PSUM accumulation.

Per-core device program (SPMD).  The default build (_build_program_raw) is
raw bass — no TileContext.  Rationale, from HW trace analysis:
  * The per-step W_hh apply is weight-load-bound: 64 LDWEIGHTS+MATMUL pairs
    issue at ~26.7 ns each (FWL fp16); the N=8 matmul streams hide under
    that.  ~1.71 us/step is the floor.
  * Under TileContext every instruction increments an engine-progress
    semaphore; those writes serialize at ~34.2 ns — slower than the 26.7 ns
    issue rate — making the semaphore unit the bottleneck (2.26 us/step).
    Raw bass pays semaphores only on real dependency edges.
  * The input projection runs as a dense prologue burst (512 N=512 matmuls):
    back-to-back streaming keeps the PE HAM-warm (2.4 GHz), ~213 ns each;
    interleaved into the recurrence they would run cold (1.2 GHz) and break
    the weight-load pipeline.  xp for all T stays resident in SBUF (8 MB),
    layout [kappa, s*4096 + c*512 + local*8 + b].
  * Recurrence MM order is staged by k-group so matmuls that consume the
    previous step's tanh-hi half issue last: [k0..3 x c0..7],
    [k4..7 x c0..3] -> tanh_lo, [k4..7 x c4..7] -> tanh_hi; tanh therefore
    overlaps the weight-load stream (~zero boundary stall).
  * The output DMA reads a DVE-staged copy of h, not the h buffer the PE
    streams from, reducing SBUF read contention with the PE's h stream.
  * inject=True replaces the two per-step identity matmuls (xp injection)
    with DVE writes of xp directly into the PSUM banks: the banks'
    has_written bits are still set from step t-3's matmuls, so the
    start=False W-MMs accumulate onto the DVE-written xp.
  Remaining known overhead: ~455 ns instruction-page fetch stall per 16 KB
  (256 instructions) of the fully-unrolled stream — a hardware-loop body
  with register addressing was tried and is ~9x slower (register-offset
  APs emit a ~170 ns patch op per instruction), so the unrolled form wins.
"""
import numpy as np

B, T, I, H = 64, 512, 1024, 1024
N_CORES = 8
BS = B // N_CORES          # 8 batches per core
NCH = H // 128             # 8 chunks of 128 along H
WIN = 64                   # steps per projection slice (512 cols)
NSLICE = T // WIN          # 8 projection slices


def _build_program(steps=T):
    from concourse import bacc, mybir
    import concourse.tile as tile

    f16 = mybir.dt.float16
    f32 = mybir.dt.float32

    nc = bacc.Bacc(None, target_bir_lowering=False)

    wih = nc.declare_dram_parameter("wih", [128, 8192], f16, isOutput=False)
    whh = nc.declare_dram_parameter("whh", [128, 8192], f16, isOutput=False)
    xt = nc.declare_dram_parameter("xt", [128, 8 * 4096], f16, isOutput=False)
    ident = nc.declare_dram_parameter("ident", [128, 128], f16, isOutput=False)
    bias = nc.declare_dram_parameter("bias", [128, 8], f32, isOutput=False)
    y = nc.declare_dram_parameter("y", [steps, 128, 64], f16, isOutput=True)

    n_slices_used = (steps + WIN - 1) // WIN

    with tile.TileContext(nc) as tc:
        with (
            tc.tile_pool(name="const", bufs=1) as const_pool,
            tc.tile_pool(name="xslice", bufs=2) as xslice_pool,
            tc.tile_pool(name="xpall", bufs=1) as xpall_pool,
            tc.tile_pool(name="hst", bufs=3) as h_pool,
            tc.tile_pool(name="pp", bufs=2, space="PSUM") as proj_psum,
            tc.tile_pool(name="rp", bufs=3, space="PSUM") as rec_psum,
        ):
            wih_sb = const_pool.tile([128, 8192], f16)
            whh_sb = const_pool.tile([128, 8192], f16)
            ident_sb = const_pool.tile([128, 128], f16)
            bias_sb = const_pool.tile([128, 8], f32)
            nc.sync.dma_start(wih_sb[:], wih[:])
            nc.sync.dma_start(whh_sb[:], whh[:])
            nc.sync.dma_start(ident_sb[:], ident[:])
            nc.sync.dma_start(bias_sb[:], bias[:])

            eng_cycle = [nc.sync, nc.gpsimd]

            def load_xt_slice(s):
                """DMA xt k-chunks for slice s into a fresh [128, 4096] tile."""
                xsl = xslice_pool.tile([128, 8 * 512], f16, name="xsl", tag="xsl")
                for k in range(8):
                    eng_cycle[k % 2].dma_start(
                        xsl[:, k * 512:(k + 1) * 512],
                        xt[:, k * 4096 + s * 512: k * 4096 + (s + 1) * 512],
                    )
                return xsl

            # ------------- phase 1: full input projection (dense burst) ------
            # xp_all layout: [kappa, s*4096 + c*512 + local_t*8 + b]
            xp_all = xpall_pool.tile([128, n_slices_used * 4096], f16)

            xsl_next = load_xt_slice(0)
            for s in range(n_slices_used):
                xsl = xsl_next
                if s + 1 < n_slices_used:
                    xsl_next = load_xt_slice(s + 1)
                for c in range(NCH):
                    psum = proj_psum.tile([128, 512], f32, name="ppsum", tag="ppsum")
                    for k in range(8):
                        nc.tensor.matmul(
                            psum[:],
                            wih_sb[:, k * 1024 + c * 128: k * 1024 + (c + 1) * 128],
                            xsl[:, k * 512:(k + 1) * 512],
                            start=(k == 0), stop=(k == 7),
                        )
                    nc.vector.tensor_scalar_add(
                        xp_all[:, s * 4096 + c * 512: s * 4096 + (c + 1) * 512],
                        psum[:],
                        bias_sb[:, c:c + 1],
                    )

            # ------------- phase 2: recurrence -------------------------------
            h_cur = None
            for t in range(steps):
                s = t // WIN
                local = t - s * WIN
                xp3 = xp_all[:, s * 4096:(s + 1) * 4096].rearrange(
                    "p (c n) -> p c n", c=NCH)

                psum_lo = rec_psum.tile([128, 4, 8], f32, name="pslo", tag="pslo")
                psum_hi = rec_psum.tile([128, 4, 8], f32, name="pshi", tag="pshi")
                nc.tensor.matmul(
                    psum_lo[:], ident_sb[:], xp3[:, 0:4, local * 8:(local + 1) * 8],
                    start=True, stop=(t == 0), skip_group_check=True)
                nc.tensor.matmul(
                    psum_hi[:], ident_sb[:], xp3[:, 4:8, local * 8:(local + 1) * 8],
                    start=True, stop=(t == 0), skip_group_check=True)

                def wblock(c, k, last):
                    pt = psum_lo if c < 4 else psum_hi
                    nc.tensor.matmul(
                        pt[:, c % 4, :],
                        whh_sb[:, k * 1024 + c * 128: k * 1024 + (c + 1) * 128],
                        h_cur[:, k * 8:(k + 1) * 8],
                        start=False, stop=last,
                        skip_group_check=True,
                    )

                h_new = h_pool.tile([128, 64], f16)
                if t > 0:
                    # needs prev tanh_lo only (h chunks 0..3)
                    for k in range(4):
                        for c in range(8):
                            wblock(c, k, False)
                    # needs prev tanh_hi (h chunks 4..7); finish psum_lo first
                    for k in range(4, 8):
                        for c in range(4):
                            wblock(c, k, (k == 7 and c == 3))
                nc.scalar.activation(
                    h_new[:, 0:32],
                    psum_lo[:].rearrange("p c n -> p (c n)"),
                    mybir.ActivationFunctionType.Tanh,
                )
                if t > 0:
                    for k in range(4, 8):
                        for c in range(4, 8):
                            wblock(c, k, (k == 7 and c == 7))
                nc.scalar.activation(
                    h_new[:, 32:64],
                    psum_hi[:].rearrange("p c n -> p (c n)"),
                    mybir.ActivationFunctionType.Tanh,
                )
                nc.sync.dma_start(y[t], h_new[:])
                h_cur = h_new

    nc.compile()
    return nc


def _build_program_raw(steps=T, inject=False):
    """Raw-bass build (no TileContext): the tile framework increments a
    per-engine progress semaphore on EVERY instruction, and those semaphore
    writes serialize at ~34 ns each — slower than the ~27 ns LDW+MM issue
    rate, making the semaphore unit the bottleneck (measured: step period
    2255 ns == 66 MMs x 34.2 ns).  Raw bass places semaphores only on the
    real dependency edges (2 psum stops + 2 tanh + DMAs per step).
    """
    from concourse import bacc, mybir
    import concourse.bass as bass

    f16 = mybir.dt.float16
    f32 = mybir.dt.float32
    Tanh = mybir.ActivationFunctionType.Tanh

    nc = bacc.Bacc(None, target_bir_lowering=False)

    wih = nc.declare_dram_parameter("wih", [128, 8192], f16, isOutput=False)
    whh = nc.declare_dram_parameter("whh", [128, 8192], f16, isOutput=False)
    xt = nc.declare_dram_parameter("xt", [128, 8 * 4096], f16, isOutput=False)
    ident = nc.declare_dram_parameter("ident", [128, 128], f16, isOutput=False)
    bias = nc.declare_dram_parameter("bias", [128, 8], f32, isOutput=False)
    y = nc.declare_dram_parameter("y", [steps, 128, 64], f16, isOutput=True)

    n_slices = (steps + WIN - 1) // WIN

    wih_sb = nc.alloc_sbuf_tensor("wih_sb", [128, 8192], f16)
    whh_sb = nc.alloc_sbuf_tensor("whh_sb", [128, 8192], f16)
    ident_sb = nc.alloc_sbuf_tensor("ident_sb", [128, 128], f16)
    bias_sb = nc.alloc_sbuf_tensor("bias_sb", [128, 8], f32)
    xp_all = nc.alloc_sbuf_tensor("xp_all", [128, n_slices * 4096], f16)
    xsl = [nc.alloc_sbuf_tensor(f"xsl{i}", [128, 4096], f16) for i in range(2)]
    hbuf = [nc.alloc_sbuf_tensor(f"hbuf{i}", [128, 64], f16) for i in range(4)]

    ystage = nc.alloc_sbuf_tensor("ystage", [128, 2 * 64], f16)

    ppsum = [nc.alloc_psum_tensor(f"ppsum{i}", [128, 512], f32) for i in range(2)]
    pslo = [nc.alloc_psum_tensor(f"pslo{i}", [128, 4, 8], f32) for i in range(3)]
    pshi = [nc.alloc_psum_tensor(f"pshi{i}", [128, 4, 8], f32) for i in range(3)]

    sem_const = nc.alloc_semaphore("sem_const")
    sem_xsl0 = nc.alloc_semaphore("sem_xsl0")
    sem_xsl1 = nc.alloc_semaphore("sem_xsl1")
    sem_proj = nc.alloc_semaphore("sem_proj")
    sem_xp = nc.alloc_semaphore("sem_xp")
    sem_pslo = nc.alloc_semaphore("sem_pslo")
    sem_pshi = nc.alloc_semaphore("sem_pshi")
    sem_hlo = nc.alloc_semaphore("sem_hlo")
    sem_hhi = nc.alloc_semaphore("sem_hhi")
    sem_stg = nc.alloc_semaphore("sem_stg")
    sem_y = nc.alloc_semaphore("sem_y")
    sem_yg = nc.alloc_semaphore("sem_yg")
    sem_xpi = nc.alloc_semaphore("sem_xpi")

    HB = 4   # hbuf depth

    with nc.Block() as block:

        @block.sync
        def _(sync):
            sync.dma_start(wih_sb[:], wih[:]).then_inc(sem_const, 16)
            sync.dma_start(whh_sb[:], whh[:]).then_inc(sem_const, 16)
            sync.dma_start(ident_sb[:], ident[:]).then_inc(sem_const, 16)
            sync.dma_start(bias_sb[:], bias[:]).then_inc(sem_const, 16)
            for s in range(n_slices):
                if s >= 2:
                    # xsl[s%2] consumed once proj of slice s-2 fully issued
                    sync.wait_ge(sem_proj, 8 * (s - 1))
                for k in (0, 2, 4, 6):
                    sync.dma_start(
                        xsl[s % 2][:, k * 512:(k + 1) * 512],
                        xt[:, k * 4096 + s * 512: k * 4096 + (s + 1) * 512],
                    ).then_inc(sem_xsl0, 16)
            for t in range(steps):
                # output DMA reads the staging copy, not hbuf, so it never
                # contends with the PE's h-operand stream on SBUF reads
                slot = t % 2
                sync.wait_ge(sem_stg, t + 1)
                sync.dma_start(
                    y[t], ystage[:, slot * 64:(slot + 1) * 64]
                ).then_inc(sem_y, 16)

        @block.gpsimd
        def _(gpsimd):
            for s in range(n_slices):
                if s >= 2:
                    gpsimd.wait_ge(sem_proj, 8 * (s - 1))
                for k in (1, 3, 5, 7):
                    gpsimd.dma_start(
                        xsl[s % 2][:, k * 512:(k + 1) * 512],
                        xt[:, k * 4096 + s * 512: k * 4096 + (s + 1) * 512],
                    ).then_inc(sem_xsl1, 16)

        @block.tensor
        def _(tensor):
            tensor.wait_ge(sem_const, 64)
            # ---- phase 1: input projection, dense warm burst ----
            for s in range(n_slices):
                tensor.wait_ge(sem_xsl0, 64 * (s + 1))
                tensor.wait_ge(sem_xsl1, 64 * (s + 1))
                for c in range(NCH):
                    idx = 8 * s + c
                    if idx >= 2:
                        tensor.wait_ge(sem_xp, idx - 1)  # ppsum ping-pong WAR
                    for k in range(8):
                        mm = tensor.matmul(
                            ppsum[idx % 2][:],
                            wih_sb[:, k * 1024 + c * 128: k * 1024 + (c + 1) * 128],
                            xsl[s % 2][:, k * 512:(k + 1) * 512],
                            start=(k == 0), stop=(k == 7),
                        )
                        if k == 7:
                            mm.then_inc(sem_proj, 1)

            # ---- phase 2: recurrence ----
            for t in range(steps):
                s = t // WIN
                local = t - s * WIN
                if local == 0:
                    tensor.wait_ge(sem_xp, 8 * (s + 1))
                lo = pslo[t % 3]
                hi = pshi[t % 3]
                if inject and t >= 3:
                    # xp was written into this psum bank by the DVE; the
                    # bank's has_written bits are still set from step t-3's
                    # matmuls, so the start=False W-MMs accumulate onto it.
                    tensor.wait_ge(sem_xpi, t - 2)
                else:
                    xp3 = xp_all[:, s * 4096:(s + 1) * 4096].rearrange(
                        "p (c n) -> p c n", c=NCH)
                    mm = tensor.matmul(
                        lo[:], ident_sb[:], xp3[:, 0:4, local * 8:(local + 1) * 8],
                        start=True, stop=(t == 0), skip_group_check=True)
                    if t == 0:
                        mm.then_inc(sem_pslo, 1)
                    mm = tensor.matmul(
                        hi[:], ident_sb[:], xp3[:, 4:8, local * 8:(local + 1) * 8],
                        start=True, stop=(t == 0), skip_group_check=True)
                    if t == 0:
                        mm.then_inc(sem_pshi, 1)

                if t == 0:
                    continue
                h_prev = hbuf[(t - 1) % HB]

                def wblock(c, k, last, sem=None):
                    pt = lo if c < 4 else hi
                    mm = tensor.matmul(
                        pt[:, c % 4, :],
                        whh_sb[:, k * 1024 + c * 128: k * 1024 + (c + 1) * 128],
                        h_prev[:, k * 8:(k + 1) * 8],
                        start=False, stop=last,
                        skip_group_check=True,
                    )
                    if sem is not None:
                        mm.then_inc(sem, 1)

                # needs h chunks 0..3 only (prev tanh_lo)
                tensor.wait_ge(sem_hlo, t)
                for k in range(4):
                    for c in range(4):
                        wblock(c, k, False)
                for k in range(2):
                    for c in range(4, 8):
                        wblock(c, k, False)
                # needs h chunks 4..7 (prev tanh_hi)
                tensor.wait_ge(sem_hhi, t)
                for k in range(4, 8):
                    for c in range(4):
                        wblock(c, k, (k == 7 and c == 3),
                               sem_pslo if (k == 7 and c == 3) else None)
                for k in range(2, 4):
                    for c in range(4, 8):
                        wblock(c, k, False)
                for k in range(4, 8):
                    for c in range(4, 8):
                        wblock(c, k, (k == 7 and c == 7),
                               sem_pshi if (k == 7 and c == 7) else None)

        @block.scalar
        def _(scalar):
            for t in range(steps):
                scalar.wait_ge(sem_pslo, t + 1)
                if t >= HB:
                    scalar.wait_ge(sem_stg, t - HB + 1)  # hbuf WAR vs stage copy
                scalar.activation(
                    hbuf[t % HB][:, 0:32],
                    pslo[t % 3][:].rearrange("p c n -> p (c n)"),
                    Tanh,
                ).then_inc(sem_hlo, 1)
                scalar.wait_ge(sem_pshi, t + 1)
                scalar.activation(
                    hbuf[t % HB][:, 32:64],
                    pshi[t % 3][:].rearrange("p c n -> p (c n)"),
                    Tanh,
                ).then_inc(sem_hhi, 1)

        @block.vector
        def _(vector):
            vector.wait_ge(sem_const, 64)
            for s in range(n_slices):
                for c in range(NCH):
                    idx = 8 * s + c
                    vector.wait_ge(sem_proj, idx + 1)
                    vector.tensor_scalar_add(
                        xp_all[:, s * 4096 + c * 512: s * 4096 + (c + 1) * 512],
                        ppsum[idx % 2][:],
                        bias_sb[:, c:c + 1],
                    ).then_inc(sem_xp, 1)
            for t in range(steps):
                slot = t % 2
                vector.wait_ge(sem_hhi, t + 1)
                if t >= 2:
                    vector.wait_ge(sem_y, 16 * (t - 1))   # stage slot WAR
                vector.tensor_copy(
                    ystage[:, slot * 64:(slot + 1) * 64], hbuf[t % HB][:]
                ).then_inc(sem_stg, 1)
                tt = t + 2
                if inject and 3 <= tt < steps:
                    # pre-write xp_tt into the psum banks of step tt; WAR vs
                    # ACT reads of bank tt%3 (= bank of tt-3) is implied by
                    # the sem_hhi wait above (ACT(t) done => ACT(tt-3) done)
                    ss, ll = tt // WIN, tt % WIN
                    src_lo = bass.AP(
                        xp_all, ss * 4096 + ll * 8,
                        [[n_slices * 4096, 128], [512, 4], [1, 8]])
                    src_hi = bass.AP(
                        xp_all, ss * 4096 + 2048 + ll * 8,
                        [[n_slices * 4096, 128], [512, 4], [1, 8]])
                    vector.tensor_copy(
                        pslo[tt % 3][:].rearrange("p c n -> p (c n)"), src_lo)
                    vector.tensor_copy(
                        pshi[tt % 3][:].rearrange("p c n -> p (c n)"), src_hi
                    ).then_inc(sem_xpi, 1)

    nc.compile()
    return nc


def _build_program_loop(steps=T):
    """Raw bass + hardware loop.

    The straight-line raw program stalls ~450-870 ns at every 16 KB
    instruction-page boundary (pc % 256 == 0): the tensor sequencer consumes
    64 B instructions at ~4.8 GB/s and the pager does not prefetch ahead.
    Fix: run the recurrence as a per-engine hardware loop whose one-step body
    (~150 instructions, <16 KB) stays resident in IRAM.  Buffer rotation
    (h/psum double-buffers, xp/y offsets, semaphore thresholds) is done with
    engine registers; all weight addresses stay constant (LDWEIGHTS cannot
    take register offsets, matmul rhs/out can).

    xp layout here is c-major: col = c*4096 + t*8 + b, so the ident-matmul rhs
    offset is linear in t (offset = t*8, hi at +16384).
    """
    from concourse import bacc, mybir
    import concourse.bass as bass

    f16 = mybir.dt.float16
    f32 = mybir.dt.float32
    Tanh = mybir.ActivationFunctionType.Tanh

    nc = bacc.Bacc(None, target_bir_lowering=False)

    wih = nc.declare_dram_parameter("wih", [128, 8192], f16, isOutput=False)
    whh = nc.declare_dram_parameter("whh", [128, 8192], f16, isOutput=False)
    xt = nc.declare_dram_parameter("xt", [128, 8 * 4096], f16, isOutput=False)
    ident = nc.declare_dram_parameter("ident", [128, 128], f16, isOutput=False)
    bias = nc.declare_dram_parameter("bias", [128, 8], f32, isOutput=False)
    y = nc.declare_dram_parameter("y", [steps, 128, 64], f16, isOutput=True)

    n_slices = (steps + WIN - 1) // WIN
    HEAD = min(8, steps)
    NLOOP = steps - HEAD

    wih_sb = nc.alloc_sbuf_tensor("wih_sb", [128, 8192], f16)
    whh_sb = nc.alloc_sbuf_tensor("whh_sb", [128, 8192], f16)
    ident_sb = nc.alloc_sbuf_tensor("ident_sb", [128, 128], f16)
    bias_sb = nc.alloc_sbuf_tensor("bias_sb", [128, 8], f32)
    xp_all = nc.alloc_sbuf_tensor("xp_all", [128, n_slices * 4096], f16)
    xsl = [nc.alloc_sbuf_tensor(f"xsl{i}", [128, 4096], f16) for i in range(2)]
    hball = nc.alloc_sbuf_tensor("hball", [128, 128], f16)   # 2 slots x 64
    ystage = nc.alloc_sbuf_tensor("ystage", [128, 64], f16)

    ppsum = [nc.alloc_psum_tensor(f"ppsum{i}", [128, 512], f32) for i in range(2)]
    pslo2 = nc.alloc_psum_tensor("pslo2", [128, 1024], f32)  # 2 slots x bank
    pshi2 = nc.alloc_psum_tensor("pshi2", [128, 1024], f32)

    sem_const = nc.alloc_semaphore("sem_const")
    sem_xsl0 = nc.alloc_semaphore("sem_xsl0")
    sem_xsl1 = nc.alloc_semaphore("sem_xsl1")
    sem_proj = nc.alloc_semaphore("sem_proj")
    sem_xp = nc.alloc_semaphore("sem_xp")
    sem_pslo = nc.alloc_semaphore("sem_pslo")
    sem_pshi = nc.alloc_semaphore("sem_pshi")
    sem_hlo = nc.alloc_semaphore("sem_hlo")
    sem_hhi = nc.alloc_semaphore("sem_hhi")
    sem_stg = nc.alloc_semaphore("sem_stg")
    sem_y = nc.alloc_semaphore("sem_y")

    # AP patterns (probed shapes; offsets in elements)
    P_XP3 = [[n_slices * 4096, 128], [4096, 4], [1, 8]]    # ident rhs view
    P_PS8 = [[1024, 128], [1, 8]]                           # one c-region
    P_PS32 = [[1024, 128], [1, 32]]                         # ident out / ACT src
    P_H8 = [[128, 128], [1, 8]]                             # one h chunk
    P_H32 = [[128, 128], [1, 32]]                           # ACT dst half
    P_H64 = [[128, 128], [1, 64]]                           # stage copy src
    P_Y = [[64, 128], [1, 64]]                              # y[t]

    def ap(tensor, off, pat):
        return bass.AP(tensor, off, pat)

    with nc.Block() as block:

        @block.sync
        def _(sync):
            sync.dma_start(wih_sb[:], wih[:]).then_inc(sem_const, 16)
            sync.dma_start(whh_sb[:], whh[:]).then_inc(sem_const, 16)
            sync.dma_start(ident_sb[:], ident[:]).then_inc(sem_const, 16)
            sync.dma_start(bias_sb[:], bias[:]).then_inc(sem_const, 16)
            for s in range(n_slices):
                if s >= 2:
                    sync.wait_ge(sem_proj, 8 * (s - 1))
                for k in (0, 2, 4, 6):
                    sync.dma_start(
                        xsl[s % 2][:, k * 512:(k + 1) * 512],
                        xt[:, k * 4096 + s * 512: k * 4096 + (s + 1) * 512],
                    ).then_inc(sem_xsl0, 16)
            for t in range(HEAD):
                sync.wait_ge(sem_stg, t + 1)
                sync.dma_start(y[t], ystage[:]).then_inc(sem_y, 16)
            if NLOOP > 0:
                y_thr = sync.alloc_register("y_thr")
                y_off = sync.alloc_register("y_off")
                sync.reg_mov(y_thr, HEAD)
                sync.reg_mov(y_off, HEAD * 8192)
                with sync.Fori(0, NLOOP):
                    sync.reg_add(y_thr, y_thr, 1)
                    sync.wait_ge(sem_stg, y_thr)
                    sync.dma_start(
                        ap(y, y_off, P_Y), ystage[:]
                    ).then_inc(sem_y, 16)
                    sync.reg_add(y_off, y_off, 8192)

        @block.gpsimd
        def _(gpsimd):
            for s in range(n_slices):
                if s >= 2:
                    gpsimd.wait_ge(sem_proj, 8 * (s - 1))
                for k in (1, 3, 5, 7):
                    gpsimd.dma_start(
                        xsl[s % 2][:, k * 512:(k + 1) * 512],
                        xt[:, k * 4096 + s * 512: k * 4096 + (s + 1) * 512],
                    ).then_inc(sem_xsl1, 16)

        @block.tensor
        def _(tensor):
            tensor.wait_ge(sem_const, 64)
            # ---- phase 1: input projection ----
            for s in range(n_slices):
                tensor.wait_ge(sem_xsl0, 64 * (s + 1))
                tensor.wait_ge(sem_xsl1, 64 * (s + 1))
                for c in range(NCH):
                    idx = 8 * s + c
                    if idx >= 2:
                        tensor.wait_ge(sem_xp, idx - 1)
                    for k in range(8):
                        mm = tensor.matmul(
                            ppsum[idx % 2][:],
                            wih_sb[:, k * 1024 + c * 128: k * 1024 + (c + 1) * 128],
                            xsl[s % 2][:, k * 512:(k + 1) * 512],
                            start=(k == 0), stop=(k == 7),
                        )
                        if k == 7:
                            mm.then_inc(sem_proj, 1)

            # ---- phase 2 helpers ----
            def w_lhsT(c, k):
                return whh_sb[:, k * 1024 + c * 128: k * 1024 + (c + 1) * 128]

            def emit_step(t, pslo_off, pshi_off, xp_off, xph_off, h_off,
                          ps_c, hk, wait_lo, wait_hi):
                """pslo_off/...: ScalarInput offsets; ps_c[c]: per-c psum
                offsets (c 0..7); hk[k]: h chunk offsets; wait_*: callables."""
                mm = tensor.matmul(
                    ap(pslo2, pslo_off, P_PS32), ident_sb[:],
                    ap(xp_all, xp_off, P_XP3),
                    start=True, stop=(t == 0), skip_group_check=True)
                if t == 0:
                    mm.then_inc(sem_pslo, 1)
                mm = tensor.matmul(
                    ap(pshi2, pshi_off, P_PS32), ident_sb[:],
                    ap(xp_all, xph_off, P_XP3),
                    start=True, stop=(t == 0), skip_group_check=True)
                if t == 0:
                    mm.then_inc(sem_pshi, 1)
                if t == 0:
                    return

                def wblock(c, k, last, sem=None):
                    pt = pslo2 if c < 4 else pshi2
                    mm = tensor.matmul(
                        ap(pt, ps_c[c], P_PS8),
                        w_lhsT(c, k),
                        ap(hball, hk[k], P_H8),
                        start=False, stop=last, skip_group_check=True)
                    if sem is not None:
                        mm.then_inc(sem, 1)

                wait_lo()
                for k in range(4):
                    for c in range(4):
                        wblock(c, k, False)
                for k in range(2):
                    for c in range(4, 8):
                        wblock(c, k, False)
                wait_hi()
                for k in range(4, 8):
                    for c in range(4):
                        wblock(c, k, (k == 7 and c == 3),
                               sem_pslo if (k == 7 and c == 3) else None)
                for k in range(2, 4):
                    for c in range(4, 8):
                        wblock(c, k, False)
                for k in range(4, 8):
                    for c in range(4, 8):
                        wblock(c, k, (k == 7 and c == 7),
                               sem_pshi if (k == 7 and c == 7) else None)

            # ---- head steps (literal addressing) ----
            for t in range(HEAD):
                s = t // WIN
                if t % WIN == 0:
                    tensor.wait_ge(sem_xp, 8 * (s + 1))
                slot = (t % 2) * 512
                hoff = ((t - 1) % 2) * 64
                emit_step(
                    t, slot, slot, t * 8, 16384 + t * 8, hoff,
                    [slot + (c % 4) * 8 for c in range(8)],
                    [hoff + k * 8 for k in range(8)],
                    (lambda tt=t: tensor.wait_ge(sem_hlo, tt)),
                    (lambda tt=t: tensor.wait_ge(sem_hhi, tt)),
                )

            # ---- loop steps (register addressing) ----
            if NLOOP > 0:
                # xp is fully materialized before t=8 only if HEAD>=... the
                # loop body cannot wait per-slice; wait for ALL slices now.
                tensor.wait_ge(sem_xp, 8 * n_slices)
                r_pslo = tensor.alloc_register("r_pslo")
                r_pshi = tensor.alloc_register("r_pshi")
                r_psc = [tensor.alloc_register(f"r_psc{j}") for j in range(1, 4)]
                r_phc = [tensor.alloc_register(f"r_phc{j}") for j in range(1, 4)]
                r_h = tensor.alloc_register("r_h")
                r_hk = [tensor.alloc_register(f"r_hk{k}") for k in range(1, 8)]
                r_xp = tensor.alloc_register("r_xp")
                r_xph = tensor.alloc_register("r_xph")
                r_tlo = tensor.alloc_register("r_tlo")
                r_thi = tensor.alloc_register("r_thi")
                # inits: flips happen at body start, so store the PREVIOUS value
                tensor.reg_mov(r_pslo, ((HEAD - 1) % 2) * 512)
                tensor.reg_mov(r_pshi, ((HEAD - 1) % 2) * 512)
                for j in range(1, 4):
                    tensor.reg_mov(r_psc[j - 1], ((HEAD - 1) % 2) * 512 + j * 8)
                    tensor.reg_mov(r_phc[j - 1], ((HEAD - 1) % 2) * 512 + j * 8)
                tensor.reg_mov(r_h, (HEAD % 2) * 64)       # flip -> (HEAD-1)%2
                for k in range(1, 8):
                    tensor.reg_mov(r_hk[k - 1], (HEAD % 2) * 64 + k * 8)
                tensor.reg_mov(r_xp, HEAD * 8)
                tensor.reg_mov(r_xph, 16384 + HEAD * 8)
                tensor.reg_mov(r_tlo, HEAD - 1)
                tensor.reg_mov(r_thi, HEAD - 1)

                with tensor.Fori(0, NLOOP):
                    tensor.reg_sub(r_pslo, 512, r_pslo)
                    tensor.reg_sub(r_pshi, 512, r_pshi)
                    for j in range(1, 4):
                        tensor.reg_sub(r_psc[j - 1], 512 + 16 * j, r_psc[j - 1])
                        tensor.reg_sub(r_phc[j - 1], 512 + 16 * j, r_phc[j - 1])
                    tensor.reg_sub(r_h, 64, r_h)
                    for k in range(1, 8):
                        tensor.reg_sub(r_hk[k - 1], 64 + 16 * k, r_hk[k - 1])
                    tensor.reg_add(r_tlo, r_tlo, 1)
                    tensor.reg_add(r_thi, r_thi, 1)
                    ps_c = ([r_pslo] + r_psc + [r_pshi] + r_phc)
                    hk = [r_h] + r_hk
                    emit_step(
                        -1, r_pslo, r_pshi, r_xp, r_xph, r_h,
                        ps_c, hk,
                        (lambda: tensor.wait_ge(sem_hlo, r_tlo)),
                        (lambda: tensor.wait_ge(sem_hhi, r_thi)),
                    )
                    tensor.reg_add(r_xp, r_xp, 8)
                    tensor.reg_add(r_xph, r_xph, 8)

        @block.scalar
        def _(scalar):
            def act_pair(pslo_off, pshi_off, hdst_lo, hdst_hi,
                         w_lo, w_stg, w_hi):
                w_lo()
                w_stg()
                scalar.activation(
                    ap(hball, hdst_lo, P_H32),
                    ap(pslo2, pslo_off, P_PS32), Tanh,
                ).then_inc(sem_hlo, 1)
                w_hi()
                scalar.activation(
                    ap(hball, hdst_hi, P_H32),
                    ap(pshi2, pshi_off, P_PS32), Tanh,
                ).then_inc(sem_hhi, 1)

            for t in range(HEAD):
                slot = (t % 2) * 512
                hd = (t % 2) * 64
                act_pair(
                    slot, slot, hd, hd + 32,
                    (lambda tt=t: scalar.wait_ge(sem_pslo, tt + 1)),
                    (lambda tt=t: scalar.wait_ge(sem_stg, tt - 1) if tt >= 2 else None),
                    (lambda tt=t: scalar.wait_ge(sem_pshi, tt + 1)),
                )
            if NLOOP > 0:
                s_ps = scalar.alloc_register("s_ps")
                s_ph = scalar.alloc_register("s_ph")
                s_hd = scalar.alloc_register("s_hd")
                s_hd32 = scalar.alloc_register("s_hd32")
                s_tlo = scalar.alloc_register("s_tlo")
                s_tst = scalar.alloc_register("s_tst")
                s_thi = scalar.alloc_register("s_thi")
                scalar.reg_mov(s_ps, ((HEAD - 1) % 2) * 512)
                scalar.reg_mov(s_ph, ((HEAD - 1) % 2) * 512)
                scalar.reg_mov(s_hd, ((HEAD - 1) % 2) * 64)
                scalar.reg_mov(s_tlo, HEAD)
                scalar.reg_mov(s_tst, HEAD - 2)
                scalar.reg_mov(s_thi, HEAD)
                with scalar.Fori(0, NLOOP):
                    scalar.reg_sub(s_ps, 512, s_ps)
                    scalar.reg_sub(s_ph, 512, s_ph)
                    scalar.reg_sub(s_hd, 64, s_hd)
                    scalar.reg_add(s_hd32, s_hd, 32)
                    scalar.reg_add(s_tlo, s_tlo, 1)
                    scalar.reg_add(s_tst, s_tst, 1)
                    scalar.reg_add(s_thi, s_thi, 1)
                    act_pair(
                        s_ps, s_ph, s_hd, s_hd32,
                        (lambda: scalar.wait_ge(sem_pslo, s_tlo)),
                        (lambda: scalar.wait_ge(sem_stg, s_tst)),
                        (lambda: scalar.wait_ge(sem_pshi, s_thi)),
                    )

        @block.vector
        def _(vector):
            vector.wait_ge(sem_const, 64)
            for s in range(n_slices):
                for c in range(NCH):
                    idx = 8 * s + c
                    vector.wait_ge(sem_proj, idx + 1)
                    vector.tensor_scalar_add(
                        xp_all[:, c * 4096 + s * 512: c * 4096 + (s + 1) * 512],
                        ppsum[idx % 2][:],
                        bias_sb[:, c:c + 1],
                    ).then_inc(sem_xp, 1)
            for t in range(HEAD):
                vector.wait_ge(sem_hhi, t + 1)
                if t >= 1:
                    vector.wait_ge(sem_y, 16 * t)
                vector.tensor_copy(
                    ystage[:], ap(hball, (t % 2) * 64, P_H64)
                ).then_inc(sem_stg, 1)
            if NLOOP > 0:
                v_thr = vector.alloc_register("v_thr")
                v_y = vector.alloc_register("v_y")
                v_hs = vector.alloc_register("v_hs")
                vector.reg_mov(v_thr, HEAD)
                vector.reg_mov(v_y, 16 * (HEAD - 1))
                vector.reg_mov(v_hs, ((HEAD - 1) % 2) * 64)
                with vector.Fori(0, NLOOP):
                    vector.reg_add(v_thr, v_thr, 1)
                    vector.wait_ge(sem_hhi, v_thr)
                    vector.reg_add(v_y, v_y, 16)
                    vector.wait_ge(sem_y, v_y)
                    vector.reg_sub(v_hs, 64, v_hs)
                    vector.tensor_copy(
                        ystage[:], ap(hball, v_hs, P_H64)
                    ).then_inc(sem_stg, 1)

    nc.compile()
    return nc


_PROGRAM_CACHE = {}
BUILD_KW = {"raw": True, "inject": True}


def _get_program(steps=T):
    key = (steps, tuple(sorted(BUILD_KW.items())))
    if key not in _PROGRAM_CACHE:
        kw = dict(BUILD_KW)
        if kw.pop("loop", False):
            builder = _build_program_loop
        elif kw.pop("raw", False):
            builder = _build_program_raw
        else:
            builder = _build_program
        _PROGRAM_CACHE[key] = builder(steps, **kw)
    return _PROGRAM_CACHE[key]


def _prep_shared(W_ih, W_hh, b_ih, b_hh):
    # lhsT layout [kappa, k*1024 + j] = W[j, k*128+kappa]
    def to_lhsT(W):
        return np.ascontiguousarray(
            W.T.reshape(8, 128, 1024).transpose(1, 0, 2).reshape(128, 8192)
        )

    wih_np = to_lhsT(np.asarray(W_ih)).astype(np.float16)
    whh_np = to_lhsT(np.asarray(W_hh)).astype(np.float16)
    bias_np = np.ascontiguousarray(
        (np.asarray(b_ih) + np.asarray(b_hh)).astype(np.float32).reshape(8, 128).T
    )
    ident_np = np.eye(128, dtype=np.float16)
    return wih_np, whh_np, bias_np, ident_np


TRACE = False
LAST_RESULT = [None]


def kernel(x, W_ih, W_hh, b_ih, b_hh, _steps=T):
    from concourse.bass_utils import run_bass_kernel_spmd

    x = np.asarray(x)
    steps = _steps
    nc = _get_program(steps)
    wih_np, whh_np, bias_np, ident_np = _prep_shared(W_ih, W_hh, b_ih, b_hh)

    in_maps = []
    for core in range(N_CORES):
        xs = x[core * BS:(core + 1) * BS]          # [8, T, I]
        # xt[kappa, k*4096 + t*8 + b] = x[b, t, k*128+kappa]
        xt_np = np.ascontiguousarray(
            xs.transpose(2, 1, 0)                   # [I, T, B]
            .reshape(8, 128, T * BS)                # [k, kappa, t*8+b]
            .transpose(1, 0, 2)                     # [kappa, k, t*8+b]
            .reshape(128, 8 * 4096)
        ).astype(np.float16)
        in_maps.append({
            "wih": wih_np, "whh": whh_np, "xt": xt_np,
            "ident": ident_np, "bias": bias_np,
        })

    res = run_bass_kernel_spmd(nc, in_maps, list(range(N_CORES)), trace=TRACE)
    LAST_RESULT[0] = res

    out = np.empty((B, T, H), dtype=np.float32)
    for core in range(N_CORES):
        yv = res.results[core]["y"]                 # [steps, 128, 64] fp16
        hb = (
            yv.reshape(steps, 128, 8, 8)
            .transpose(3, 0, 2, 1)                  # [b, t, c, kappa]
            .reshape(BS, steps, H)
            .astype(np.float32)
        )
        out[core * BS:(core + 1) * BS, :steps] = hb
    return out


# revision 22
# speedup vs baseline: 8.8117x; 1.0060x over previous
"""Trainium2 Bass kernel for nn_BasicRNNBlock (vanilla tanh RNN).

Reference semantics (fp32):
    xp = einsum("bti,hi->tbh", x, W_ih) + b_ih + b_hh      # input projection
    h_t = tanh(xp_t + h_{t-1} @ W_hh.T),  h_0 = 0          # T sequential steps
    out[b, t, :] = h_t[b]                                  # [B, T, H]

Shapes: B=64, T=512, I=H=1024.  Sharding: data-parallel over batch across
8 NeuronCores (8 batches/core, weights replicated).  All-fp16 matmul inputs,
fp32 PSUM accumulation.

Per-core device program (SPMD).  The default build (_build_program_raw) is
raw bass — no TileContext.  Rationale, from HW trace analysis:
  * The per-step W_hh apply is weight-load-bound: 64 LDWEIGHTS+MATMUL pairs
    issue at ~26.7 ns each (FWL fp16); the N=8 matmul streams hide under
    that.  ~1.71 us/step is the floor.
  * Under TileContext every instruction increments an engine-progress
    semaphore; those writes serialize at ~34.2 ns — slower than the 26.7 ns
    issue rate — making the semaphore unit the bottleneck (2.26 us/step).
    Raw bass pays semaphores only on real dependency edges.
  * The input projection runs as a dense prologue burst (512 N=512 matmuls):
    back-to-back streaming keeps the PE HAM-warm (2.4 GHz), ~213 ns each;
    interleaved into the recurrence they would run cold (1.2 GHz) and break
    the weight-load pipeline.  xp for all T stays resident in SBUF (8 MB),
    layout [kappa, s*4096 + c*512 + local*8 + b].
  * Recurrence MM order is staged by k-group so matmuls that consume the
    previous step's tanh-hi half issue last: [k0..3 x c0..7],
    [k4..7 x c0..3] -> tanh_lo, [k4..7 x c4..7] -> tanh_hi; tanh therefore
    overlaps the weight-load stream (~zero boundary stall).
  * The output DMA reads a DVE-staged copy of h, not the h buffer the PE
    streams from, reducing SBUF read contention with the PE's h stream.
  * inject=True replaces the two per-step identity matmuls (xp injection)
    with DVE writes of xp directly into the PSUM banks: the banks'
    has_written bits are still set from step t-3's matmuls, so the
    start=False W-MMs accumulate onto the DVE-written xp.
  Remaining known overhead: ~455 ns instruction-page fetch stall per 16 KB
  (256 instructions) of the fully-unrolled stream — a hardware-loop body
  with register addressing was tried and is ~9x slower (register-offset
  APs emit a ~170 ns patch op per instruction), so the unrolled form wins.
"""
import numpy as np

B, T, I, H = 64, 512, 1024, 1024
N_CORES = 8
BS = B // N_CORES          # 8 batches per core
NCH = H // 128             # 8 chunks of 128 along H
WIN = 64                   # steps per projection slice (512 cols)
NSLICE = T // WIN          # 8 projection slices


def _build_program(steps=T):
    from concourse import bacc, mybir
    import concourse.tile as tile

    f16 = mybir.dt.float16
    f32 = mybir.dt.float32

    nc = bacc.Bacc(None, target_bir_lowering=False)

    wih = nc.declare_dram_parameter("wih", [128, 8192], f16, isOutput=False)
    whh = nc.declare_dram_parameter("whh", [128, 8192], f16, isOutput=False)
    xt = nc.declare_dram_parameter("xt", [128, 8 * 4096], f16, isOutput=False)
    ident = nc.declare_dram_parameter("ident", [128, 128], f16, isOutput=False)
    bias = nc.declare_dram_parameter("bias", [128, 8], f32, isOutput=False)
    y = nc.declare_dram_parameter("y", [steps, 128, 64], f16, isOutput=True)

    n_slices_used = (steps + WIN - 1) // WIN

    with tile.TileContext(nc) as tc:
        with (
            tc.tile_pool(name="const", bufs=1) as const_pool,
            tc.tile_pool(name="xslice", bufs=2) as xslice_pool,
            tc.tile_pool(name="xpall", bufs=1) as xpall_pool,
            tc.tile_pool(name="hst", bufs=3) as h_pool,
            tc.tile_pool(name="pp", bufs=2, space="PSUM") as proj_psum,
            tc.tile_pool(name="rp", bufs=3, space="PSUM") as rec_psum,
        ):
            wih_sb = const_pool.tile([128, 8192], f16)
            whh_sb = const_pool.tile([128, 8192], f16)
            ident_sb = const_pool.tile([128, 128], f16)
            bias_sb = const_pool.tile([128, 8], f32)
            nc.sync.dma_start(wih_sb[:], wih[:])
            nc.sync.dma_start(whh_sb[:], whh[:])
            nc.sync.dma_start(ident_sb[:], ident[:])
            nc.sync.dma_start(bias_sb[:], bias[:])

            eng_cycle = [nc.sync, nc.gpsimd]

            def load_xt_slice(s):
                """DMA xt k-chunks for slice s into a fresh [128, 4096] tile."""
                xsl = xslice_pool.tile([128, 8 * 512], f16, name="xsl", tag="xsl")
                for k in range(8):
                    eng_cycle[k % 2].dma_start(
                        xsl[:, k * 512:(k + 1) * 512],
                        xt[:, k * 4096 + s * 512: k * 4096 + (s + 1) * 512],
                    )
                return xsl

            # ------------- phase 1: full input projection (dense burst) ------
            # xp_all layout: [kappa, s*4096 + c*512 + local_t*8 + b]
            xp_all = xpall_pool.tile([128, n_slices_used * 4096], f16)

            xsl_next = load_xt_slice(0)
            for s in range(n_slices_used):
                xsl = xsl_next
                if s + 1 < n_slices_used:
                    xsl_next = load_xt_slice(s + 1)
                for c in range(NCH):
                    psum = proj_psum.tile([128, 512], f32, name="ppsum", tag="ppsum")
                    for k in range(8):
                        nc.tensor.matmul(
                            psum[:],
                            wih_sb[:, k * 1024 + c * 128: k * 1024 + (c + 1) * 128],
                            xsl[:, k * 512:(k + 1) * 512],
                            start=(k == 0), stop=(k == 7),
                        )
                    nc.vector.tensor_scalar_add(
                        xp_all[:, s * 4096 + c * 512: s * 4096 + (c + 1) * 512],
                        psum[:],
                        bias_sb[:, c:c + 1],
                    )

            # ------------- phase 2: recurrence -------------------------------
            h_cur = None
            for t in range(steps):
                s = t // WIN
                local = t - s * WIN
                xp3 = xp_all[:, s * 4096:(s + 1) * 4096].rearrange(
                    "p (c n) -> p c n", c=NCH)

                psum_lo = rec_psum.tile([128, 4, 8], f32, name="pslo", tag="pslo")
                psum_hi = rec_psum.tile([128, 4, 8], f32, name="pshi", tag="pshi")
                nc.tensor.matmul(
                    psum_lo[:], ident_sb[:], xp3[:, 0:4, local * 8:(local + 1) * 8],
                    start=True, stop=(t == 0), skip_group_check=True)
                nc.tensor.matmul(
                    psum_hi[:], ident_sb[:], xp3[:, 4:8, local * 8:(local + 1) * 8],
                    start=True, stop=(t == 0), skip_group_check=True)

                def wblock(c, k, last):
                    pt = psum_lo if c < 4 else psum_hi
                    nc.tensor.matmul(
                        pt[:, c % 4, :],
                        whh_sb[:, k * 1024 + c * 128: k * 1024 + (c + 1) * 128],
                        h_cur[:, k * 8:(k + 1) * 8],
                        start=False, stop=last,
                        skip_group_check=True,
                    )

                h_new = h_pool.tile([128, 64], f16)
                if t > 0:
                    # needs prev tanh_lo only (h chunks 0..3)
                    for k in range(4):
                        for c in range(8):
                            wblock(c, k, False)
                    # needs prev tanh_hi (h chunks 4..7); finish psum_lo first
                    for k in range(4, 8):
                        for c in range(4):
                            wblock(c, k, (k == 7 and c == 3))
                nc.scalar.activation(
                    h_new[:, 0:32],
                    psum_lo[:].rearrange("p c n -> p (c n)"),
                    mybir.ActivationFunctionType.Tanh,
                )
                if t > 0:
                    for k in range(4, 8):
                        for c in range(4, 8):
                            wblock(c, k, (k == 7 and c == 7))
                nc.scalar.activation(
                    h_new[:, 32:64],
                    psum_hi[:].rearrange("p c n -> p (c n)"),
                    mybir.ActivationFunctionType.Tanh,
                )
                nc.sync.dma_start(y[t], h_new[:])
                h_cur = h_new

    nc.compile()
    return nc


def _build_program_raw(steps=T, inject=False):
    """Raw-bass build (no TileContext): the tile framework increments a
    per-engine progress semaphore on EVERY instruction, and those semaphore
    writes serialize at ~34 ns each — slower than the ~27 ns LDW+MM issue
    rate, making the semaphore unit the bottleneck (measured: step period
    2255 ns == 66 MMs x 34.2 ns).  Raw bass places semaphores only on the
    real dependency edges (2 psum stops + 2 tanh + DMAs per step).
    """
    from concourse import bacc, mybir
    import concourse.bass as bass

    f16 = mybir.dt.float16
    f32 = mybir.dt.float32
    Tanh = mybir.ActivationFunctionType.Tanh

    nc = bacc.Bacc(None, target_bir_lowering=False)

    wih = nc.declare_dram_parameter("wih", [128, 8192], f16, isOutput=False)
    whh = nc.declare_dram_parameter("whh", [128, 8192], f16, isOutput=False)
    xt = nc.declare_dram_parameter("xt", [128, 8 * 4096], f16, isOutput=False)
    ident = nc.declare_dram_parameter("ident", [128, 128], f16, isOutput=False)
    bias = nc.declare_dram_parameter("bias", [128, 8], f32, isOutput=False)
    y = nc.declare_dram_parameter("y", [steps, 128, 64], f16, isOutput=True)

    n_slices = (steps + WIN - 1) // WIN

    wih_sb = nc.alloc_sbuf_tensor("wih_sb", [128, 8192], f16)
    whh_sb = nc.alloc_sbuf_tensor("whh_sb", [128, 8192], f16)
    ident_sb = nc.alloc_sbuf_tensor("ident_sb", [128, 128], f16)
    bias_sb = nc.alloc_sbuf_tensor("bias_sb", [128, 8], f32)
    xp_all = nc.alloc_sbuf_tensor("xp_all", [128, n_slices * 4096], f16)
    xsl = [nc.alloc_sbuf_tensor(f"xsl{i}", [128, 4096], f16) for i in range(2)]
    hbuf = [nc.alloc_sbuf_tensor(f"hbuf{i}", [128, 64], f16) for i in range(4)]

    ystage = nc.alloc_sbuf_tensor("ystage", [128, 2 * 64], f16)

    ppsum = [nc.alloc_psum_tensor(f"ppsum{i}", [128, 512], f32) for i in range(2)]
    pslo = [nc.alloc_psum_tensor(f"pslo{i}", [128, 4, 8], f32) for i in range(3)]
    pshi = [nc.alloc_psum_tensor(f"pshi{i}", [128, 4, 8], f32) for i in range(3)]

    sem_const = nc.alloc_semaphore("sem_const")
    sem_xsl0 = nc.alloc_semaphore("sem_xsl0")
    sem_xsl1 = nc.alloc_semaphore("sem_xsl1")
    sem_proj = nc.alloc_semaphore("sem_proj")
    sem_xp = nc.alloc_semaphore("sem_xp")
    sem_pslo = nc.alloc_semaphore("sem_pslo")
    sem_pshi = nc.alloc_semaphore("sem_pshi")
    sem_hlo = nc.alloc_semaphore("sem_hlo")
    sem_hhi = nc.alloc_semaphore("sem_hhi")
    sem_stg = nc.alloc_semaphore("sem_stg")
    sem_y = nc.alloc_semaphore("sem_y")
    sem_yg = nc.alloc_semaphore("sem_yg")
    sem_xpi = nc.alloc_semaphore("sem_xpi")

    HB = 4   # hbuf depth

    with nc.Block() as block:

        @block.sync
        def _(sync):
            sync.dma_start(wih_sb[:], wih[:]).then_inc(sem_const, 16)
            sync.dma_start(whh_sb[:], whh[:]).then_inc(sem_const, 16)
            sync.dma_start(ident_sb[:], ident[:]).then_inc(sem_const, 16)
            sync.dma_start(bias_sb[:], bias[:]).then_inc(sem_const, 16)
            for s in range(n_slices):
                if s >= 2:
                    # xsl[s%2] consumed once proj of slice s-2 fully issued
                    sync.wait_ge(sem_proj, 8 * (s - 1))
                for k in (0, 2, 4, 6):
                    sync.dma_start(
                        xsl[s % 2][:, k * 512:(k + 1) * 512],
                        xt[:, k * 4096 + s * 512: k * 4096 + (s + 1) * 512],
                    ).then_inc(sem_xsl0, 16)
            for t in range(steps):
                # output DMA reads the staging copy, not hbuf, so it never
                # contends with the PE's h-operand stream on SBUF reads
                slot = t % 2
                sync.wait_ge(sem_stg, t + 1)
                sync.dma_start(
                    y[t], ystage[:, slot * 64:(slot + 1) * 64]
                ).then_inc(sem_y, 16)

        @block.gpsimd
        def _(gpsimd):
            for s in range(n_slices):
                if s >= 2:
                    gpsimd.wait_ge(sem_proj, 8 * (s - 1))
                for k in (1, 3, 5, 7):
                    gpsimd.dma_start(
                        xsl[s % 2][:, k * 512:(k + 1) * 512],
                        xt[:, k * 4096 + s * 512: k * 4096 + (s + 1) * 512],
                    ).then_inc(sem_xsl1, 16)

        @block.tensor
        def _(tensor):
            tensor.wait_ge(sem_const, 64)
            # ---- phase 1: input projection, dense warm burst ----
            for s in range(n_slices):
                tensor.wait_ge(sem_xsl0, 64 * (s + 1))
                tensor.wait_ge(sem_xsl1, 64 * (s + 1))
                for c in range(NCH):
                    idx = 8 * s + c
                    if idx >= 2:
                        tensor.wait_ge(sem_xp, idx - 1)  # ppsum ping-pong WAR
                    for k in range(8):
                        mm = tensor.matmul(
                            ppsum[idx % 2][:],
                            wih_sb[:, k * 1024 + c * 128: k * 1024 + (c + 1) * 128],
                            xsl[s % 2][:, k * 512:(k + 1) * 512],
                            start=(k == 0), stop=(k == 7),
                        )
                        if k == 7:
                            mm.then_inc(sem_proj, 1)

            # ---- phase 2: recurrence ----
            for t in range(steps):
                s = t // WIN
                local = t - s * WIN
                if local == 0:
                    tensor.wait_ge(sem_xp, 8 * (s + 1))
                lo = pslo[t % 3]
                hi = pshi[t % 3]
                if inject and t >= 3:
                    # xp was written into this psum bank by the DVE; the
                    # bank's has_written bits are still set from step t-3's
                    # matmuls, so the start=False W-MMs accumulate onto it.
                    tensor.wait_ge(sem_xpi, t - 2)
                else:
                    xp3 = xp_all[:, s * 4096:(s + 1) * 4096].rearrange(
                        "p (c n) -> p c n", c=NCH)
                    mm = tensor.matmul(
                        lo[:], ident_sb[:], xp3[:, 0:4, local * 8:(local + 1) * 8],
                        start=True, stop=(t == 0), skip_group_check=True)
                    if t == 0:
                        mm.then_inc(sem_pslo, 1)
                    mm = tensor.matmul(
                        hi[:], ident_sb[:], xp3[:, 4:8, local * 8:(local + 1) * 8],
                        start=True, stop=(t == 0), skip_group_check=True)
                    if t == 0:
                        mm.then_inc(sem_pshi, 1)

                if t == 0:
                    continue
                h_prev = hbuf[(t - 1) % HB]

                def wblock(c, k, last, sem=None):
                    pt = lo if c < 4 else hi
                    mm = tensor.matmul(
                        pt[:, c % 4, :],
                        whh_sb[:, k * 1024 + c * 128: k * 1024 + (c + 1) * 128],
                        h_prev[:, k * 8:(k + 1) * 8],
                        start=False, stop=last,
                        skip_group_check=True,
                    )
                    if sem is not None:
                        mm.then_inc(sem, 1)

                # needs h chunks 0..3 only (prev tanh_lo)
                tensor.wait_ge(sem_hlo, t)
                for k in range(4):
                    for c in range(4):
                        wblock(c, k, False)
                for k in range(2):
                    for c in range(4, 8):
                        wblock(c, k, False)
                # needs h chunks 4..7 (prev tanh_hi)
                tensor.wait_ge(sem_hhi, t)
                for k in range(4, 8):
                    for c in range(4):
                        wblock(c, k, (k == 7 and c == 3),
                               sem_pslo if (k == 7 and c == 3) else None)
                for k in range(2, 4):
                    for c in range(4, 8):
                        wblock(c, k, False)
                for k in range(4, 8):
                    for c in range(4, 8):
                        wblock(c, k, (k == 7 and c == 7),
                               sem_pshi if (k == 7 and c == 7) else None)

        @block.scalar
        def _(scalar):
            for t in range(steps):
                scalar.wait_ge(sem_pslo, t + 1)
                if t >= HB:
                    scalar.wait_ge(sem_stg, t - HB + 1)  # hbuf WAR vs stage copy
                scalar.activation(
                    hbuf[t % HB][:, 0:32],
                    pslo[t % 3][:].rearrange("p c n -> p (c n)"),
                    Tanh,
                ).then_inc(sem_hlo, 1)
                scalar.wait_ge(sem_pshi, t + 1)
                scalar.activation(
                    hbuf[t % HB][:, 32:64],
                    pshi[t % 3][:].rearrange("p c n -> p (c n)"),
                    Tanh,
                ).then_inc(sem_hhi, 1)

        @block.vector
        def _(vector):
            vector.wait_ge(sem_const, 64)
            for s in range(n_slices):
                for c in range(NCH):
                    idx = 8 * s + c
                    vector.wait_ge(sem_proj, idx + 1)
                    vector.tensor_scalar_add(
                        xp_all[:, s * 4096 + c * 512: s * 4096 + (c + 1) * 512],
                        ppsum[idx % 2][:],
                        bias_sb[:, c:c + 1],
                    ).then_inc(sem_xp, 1)
            for t in range(steps):
                slot = t % 2
                vector.wait_ge(sem_hhi, t + 1)
                if t >= 2:
                    vector.wait_ge(sem_y, 16 * (t - 1))   # stage slot WAR
                vector.tensor_copy(
                    ystage[:, slot * 64:(slot + 1) * 64], hbuf[t % HB][:]
                ).then_inc(sem_stg, 1)
                tt = t + 2
                if inject and 3 <= tt < steps:
                    # pre-write xp_tt into the psum banks of step tt; WAR vs
                    # ACT reads of bank tt%3 (= bank of tt-3) is implied by
                    # the sem_hhi wait above (ACT(t) done => ACT(tt-3) done)
                    ss, ll = tt // WIN, tt % WIN
                    src_lo = bass.AP(
                        xp_all, ss * 4096 + ll * 8,
                        [[n_slices * 4096, 128], [512, 4], [1, 8]])
                    src_hi = bass.AP(
                        xp_all, ss * 4096 + 2048 + ll * 8,
                        [[n_slices * 4096, 128], [512, 4], [1, 8]])
                    vector.tensor_copy(
                        pslo[tt % 3][:].rearrange("p c n -> p (c n)"), src_lo)
                    vector.tensor_copy(
                        pshi[tt % 3][:].rearrange("p c n -> p (c n)"), src_hi
                    ).then_inc(sem_xpi, 1)

    nc.compile()
    return nc


def _build_program_loop(steps=T):
    """Raw bass + hardware loop.

    The straight-line raw program stalls ~450-870 ns at every 16 KB
    instruction-page boundary (pc % 256 == 0): the tensor sequencer consumes
    64 B instructions at ~4.8 GB/s and the pager does not prefetch ahead.
    Fix: run the recurrence as a per-engine hardware loop whose one-step body
    (~150 instructions, <16 KB) stays resident in IRAM.  Buffer rotation
    (h/psum double-buffers, xp/y offsets, semaphore thresholds) is done with
    engine registers; all weight addresses stay constant (LDWEIGHTS cannot
    take register offsets, matmul rhs/out can).

    xp layout here is c-major: col = c*4096 + t*8 + b, so the ident-matmul rhs
    offset is linear in t (offset = t*8, hi at +16384).
    """
    from concourse import bacc, mybir
    import concourse.bass as bass

    f16 = mybir.dt.float16
    f32 = mybir.dt.float32
    Tanh = mybir.ActivationFunctionType.Tanh

    nc = bacc.Bacc(None, target_bir_lowering=False)

    wih = nc.declare_dram_parameter("wih", [128, 8192], f16, isOutput=False)
    whh = nc.declare_dram_parameter("whh", [128, 8192], f16, isOutput=False)
    xt = nc.declare_dram_parameter("xt", [128, 8 * 4096], f16, isOutput=False)
    ident = nc.declare_dram_parameter("ident", [128, 128], f16, isOutput=False)
    bias = nc.declare_dram_parameter("bias", [128, 8], f32, isOutput=False)
    y = nc.declare_dram_parameter("y", [steps, 128, 64], f16, isOutput=True)

    n_slices = (steps + WIN - 1) // WIN
    HEAD = min(8, steps)
    NLOOP = steps - HEAD

    wih_sb = nc.alloc_sbuf_tensor("wih_sb", [128, 8192], f16)
    whh_sb = nc.alloc_sbuf_tensor("whh_sb", [128, 8192], f16)
    ident_sb = nc.alloc_sbuf_tensor("ident_sb", [128, 128], f16)
    bias_sb = nc.alloc_sbuf_tensor("bias_sb", [128, 8], f32)
    xp_all = nc.alloc_sbuf_tensor("xp_all", [128, n_slices * 4096], f16)
    xsl = [nc.alloc_sbuf_tensor(f"xsl{i}", [128, 4096], f16) for i in range(2)]
    hball = nc.alloc_sbuf_tensor("hball", [128, 128], f16)   # 2 slots x 64
    ystage = nc.alloc_sbuf_tensor("ystage", [128, 64], f16)

    ppsum = [nc.alloc_psum_tensor(f"ppsum{i}", [128, 512], f32) for i in range(2)]
    pslo2 = nc.alloc_psum_tensor("pslo2", [128, 1024], f32)  # 2 slots x bank
    pshi2 = nc.alloc_psum_tensor("pshi2", [128, 1024], f32)

    sem_const = nc.alloc_semaphore("sem_const")
    sem_xsl0 = nc.alloc_semaphore("sem_xsl0")
    sem_xsl1 = nc.alloc_semaphore("sem_xsl1")
    sem_proj = nc.alloc_semaphore("sem_proj")
    sem_xp = nc.alloc_semaphore("sem_xp")
    sem_pslo = nc.alloc_semaphore("sem_pslo")
    sem_pshi = nc.alloc_semaphore("sem_pshi")
    sem_hlo = nc.alloc_semaphore("sem_hlo")
    sem_hhi = nc.alloc_semaphore("sem_hhi")
    sem_stg = nc.alloc_semaphore("sem_stg")
    sem_y = nc.alloc_semaphore("sem_y")

    # AP patterns (probed shapes; offsets in elements)
    P_XP3 = [[n_slices * 4096, 128], [4096, 4], [1, 8]]    # ident rhs view
    P_PS8 = [[1024, 128], [1, 8]]                           # one c-region
    P_PS32 = [[1024, 128], [1, 32]]                         # ident out / ACT src
    P_H8 = [[128, 128], [1, 8]]                             # one h chunk
    P_H32 = [[128, 128], [1, 32]]                           # ACT dst half
    P_H64 = [[128, 128], [1, 64]]                           # stage copy src
    P_Y = [[64, 128], [1, 64]]                              # y[t]

    def ap(tensor, off, pat):
        return bass.AP(tensor, off, pat)

    with nc.Block() as block:

        @block.sync
        def _(sync):
            sync.dma_start(wih_sb[:], wih[:]).then_inc(sem_const, 16)
            sync.dma_start(whh_sb[:], whh[:]).then_inc(sem_const, 16)
            sync.dma_start(ident_sb[:], ident[:]).then_inc(sem_const, 16)
            sync.dma_start(bias_sb[:], bias[:]).then_inc(sem_const, 16)
            for s in range(n_slices):
                if s >= 2:
                    sync.wait_ge(sem_proj, 8 * (s - 1))
                for k in (0, 2, 4, 6):
                    sync.dma_start(
                        xsl[s % 2][:, k * 512:(k + 1) * 512],
                        xt[:, k * 4096 + s * 512: k * 4096 + (s + 1) * 512],
                    ).then_inc(sem_xsl0, 16)
            for t in range(HEAD):
                sync.wait_ge(sem_stg, t + 1)
                sync.dma_start(y[t], ystage[:]).then_inc(sem_y, 16)
            if NLOOP > 0:
                y_thr = sync.alloc_register("y_thr")
                y_off = sync.alloc_register("y_off")
                sync.reg_mov(y_thr, HEAD)
                sync.reg_mov(y_off, HEAD * 8192)
                with sync.Fori(0, NLOOP):
                    sync.reg_add(y_thr, y_thr, 1)
                    sync.wait_ge(sem_stg, y_thr)
                    sync.dma_start(
                        ap(y, y_off, P_Y), ystage[:]
                    ).then_inc(sem_y, 16)
                    sync.reg_add(y_off, y_off, 8192)

        @block.gpsimd
        def _(gpsimd):
            for s in range(n_slices):
                if s >= 2:
                    gpsimd.wait_ge(sem_proj, 8 * (s - 1))
                for k in (1, 3, 5, 7):
                    gpsimd.dma_start(
                        xsl[s % 2][:, k * 512:(k + 1) * 512],
                        xt[:, k * 4096 + s * 512: k * 4096 + (s + 1) * 512],
                    ).then_inc(sem_xsl1, 16)

        @block.tensor
        def _(tensor):
            tensor.wait_ge(sem_const, 64)
            # ---- phase 1: input projection ----
            for s in range(n_slices):
                tensor.wait_ge(sem_xsl0, 64 * (s + 1))
                tensor.wait_ge(sem_xsl1, 64 * (s + 1))
                for c in range(NCH):
                    idx = 8 * s + c
                    if idx >= 2:
                        tensor.wait_ge(sem_xp, idx - 1)
                    for k in range(8):
                        mm = tensor.matmul(
                            ppsum[idx % 2][:],
                            wih_sb[:, k * 1024 + c * 128: k * 1024 + (c + 1) * 128],
                            xsl[s % 2][:, k * 512:(k + 1) * 512],
                            start=(k == 0), stop=(k == 7),
                        )
                        if k == 7:
                            mm.then_inc(sem_proj, 1)

            # ---- phase 2 helpers ----
            def w_lhsT(c, k):
                return whh_sb[:, k * 1024 + c * 128: k * 1024 + (c + 1) * 128]

            def emit_step(t, pslo_off, pshi_off, xp_off, xph_off, h_off,
                          ps_c, hk, wait_lo, wait_hi):
                """pslo_off/...: ScalarInput offsets; ps_c[c]: per-c psum
                offsets (c 0..7); hk[k]: h chunk offsets; wait_*: callables."""
                mm = tensor.matmul(
                    ap(pslo2, pslo_off, P_PS32), ident_sb[:],
                    ap(xp_all, xp_off, P_XP3),
                    start=True, stop=(t == 0), skip_group_check=True)
                if t == 0:
                    mm.then_inc(sem_pslo, 1)
                mm = tensor.matmul(
                    ap(pshi2, pshi_off, P_PS32), ident_sb[:],
                    ap(xp_all, xph_off, P_XP3),
                    start=True, stop=(t == 0), skip_group_check=True)
                if t == 0:
                    mm.then_inc(sem_pshi, 1)
                if t == 0:
                    return

                def wblock(c, k, last, sem=None):
                    pt = pslo2 if c < 4 else pshi2
                    mm = tensor.matmul(
                        ap(pt, ps_c[c], P_PS8),
                        w_lhsT(c, k),
                        ap(hball, hk[k], P_H8),
                        start=False, stop=last, skip_group_check=True)
                    if sem is not None:
                        mm.then_inc(sem, 1)

                wait_lo()
                for k in range(4):
                    for c in range(4):
                        wblock(c, k, False)
                for k in range(2):
                    for c in range(4, 8):
                        wblock(c, k, False)
                wait_hi()
                for k in range(4, 8):
                    for c in range(4):
                        wblock(c, k, (k == 7 and c == 3),
                               sem_pslo if (k == 7 and c == 3) else None)
                for k in range(2, 4):
                    for c in range(4, 8):
                        wblock(c, k, False)
                for k in range(4, 8):
                    for c in range(4, 8):
                        wblock(c, k, (k == 7 and c == 7),
                               sem_pshi if (k == 7 and c == 7) else None)

            # ---- head steps (literal addressing) ----
            for t in range(HEAD):
                s = t // WIN
                if t % WIN == 0:
                    tensor.wait_ge(sem_xp, 8 * (s + 1))
                slot = (t % 2) * 512
                hoff = ((t - 1) % 2) * 64
                emit_step(
                    t, slot, slot, t * 8, 16384 + t * 8, hoff,
                    [slot + (c % 4) * 8 for c in range(8)],
                    [hoff + k * 8 for k in range(8)],
                    (lambda tt=t: tensor.wait_ge(sem_hlo, tt)),
                    (lambda tt=t: tensor.wait_ge(sem_hhi, tt)),
                )

            # ---- loop steps (register addressing) ----
            if NLOOP > 0:
                # xp is fully materialized before t=8 only if HEAD>=... the
                # loop body cannot wait per-slice; wait for ALL slices now.
                tensor.wait_ge(sem_xp, 8 * n_slices)
                r_pslo = tensor.alloc_register("r_pslo")
                r_pshi = tensor.alloc_register("r_pshi")
                r_psc = [tensor.alloc_register(f"r_psc{j}") for j in range(1, 4)]
                r_phc = [tensor.alloc_register(f"r_phc{j}") for j in range(1, 4)]
                r_h = tensor.alloc_register("r_h")
                r_hk = [tensor.alloc_register(f"r_hk{k}") for k in range(1, 8)]
                r_xp = tensor.alloc_register("r_xp")
                r_xph = tensor.alloc_register("r_xph")
                r_tlo = tensor.alloc_register("r_tlo")
                r_thi = tensor.alloc_register("r_thi")
                # inits: flips happen at body start, so store the PREVIOUS value
                tensor.reg_mov(r_pslo, ((HEAD - 1) % 2) * 512)
                tensor.reg_mov(r_pshi, ((HEAD - 1) % 2) * 512)
                for j in range(1, 4):
                    tensor.reg_mov(r_psc[j - 1], ((HEAD - 1) % 2) * 512 + j * 8)
                    tensor.reg_mov(r_phc[j - 1], ((HEAD - 1) % 2) * 512 + j * 8)
                tensor.reg_mov(r_h, (HEAD % 2) * 64)       # flip -> (HEAD-1)%2
                for k in range(1, 8):
                    tensor.reg_mov(r_hk[k - 1], (HEAD % 2) * 64 + k * 8)
                tensor.reg_mov(r_xp, HEAD * 8)
                tensor.reg_mov(r_xph, 16384 + HEAD * 8)
                tensor.reg_mov(r_tlo, HEAD - 1)
                tensor.reg_mov(r_thi, HEAD - 1)

                with tensor.Fori(0, NLOOP):
                    tensor.reg_sub(r_pslo, 512, r_pslo)
                    tensor.reg_sub(r_pshi, 512, r_pshi)
                    for j in range(1, 4):
                        tensor.reg_sub(r_psc[j - 1], 512 + 16 * j, r_psc[j - 1])
                        tensor.reg_sub(r_phc[j - 1], 512 + 16 * j, r_phc[j - 1])
                    tensor.reg_sub(r_h, 64, r_h)
                    for k in range(1, 8):
                        tensor.reg_sub(r_hk[k - 1], 64 + 16 * k, r_hk[k - 1])
                    tensor.reg_add(r_tlo, r_tlo, 1)
                    tensor.reg_add(r_thi, r_thi, 1)
                    ps_c = ([r_pslo] + r_psc + [r_pshi] + r_phc)
                    hk = [r_h] + r_hk
                    emit_step(
                        -1, r_pslo, r_pshi, r_xp, r_xph, r_h,
                        ps_c, hk,
                        (lambda: tensor.wait_ge(sem_hlo, r_tlo)),
                        (lambda: tensor.wait_ge(sem_hhi, r_thi)),
                    )
                    tensor.reg_add(r_xp, r_xp, 8)
                    tensor.reg_add(r_xph, r_xph, 8)

        @block.scalar
        def _(scalar):
            def act_pair(pslo_off, pshi_off, hdst_lo, hdst_hi,
                         w_lo, w_stg, w_hi):
                w_lo()
                w_stg()
                scalar.activation(
                    ap(hball, hdst_lo, P_H32),
                    ap(pslo2, pslo_off, P_PS32), Tanh,
                ).then_inc(sem_hlo, 1)
                w_hi()
                scalar.activation(
                    ap(hball, hdst_hi, P_H32),
                    ap(pshi2, pshi_off, P_PS32), Tanh,
                ).then_inc(sem_hhi, 1)

            for t in range(HEAD):
                slot = (t % 2) * 512
                hd = (t % 2) * 64
                act_pair(
                    slot, slot, hd, hd + 32,
                    (lambda tt=t: scalar.wait_ge(sem_pslo, tt + 1)),
                    (lambda tt=t: scalar.wait_ge(sem_stg, tt - 1) if tt >= 2 else None),
                    (lambda tt=t: scalar.wait_ge(sem_pshi, tt + 1)),
                )
            if NLOOP > 0:
                s_ps = scalar.alloc_register("s_ps")
                s_ph = scalar.alloc_register("s_ph")
                s_hd = scalar.alloc_register("s_hd")
                s_hd32 = scalar.alloc_register("s_hd32")
                s_tlo = scalar.alloc_register("s_tlo")
                s_tst = scalar.alloc_register("s_tst")
                s_thi = scalar.alloc_register("s_thi")
                scalar.reg_mov(s_ps, ((HEAD - 1) % 2) * 512)
                scalar.reg_mov(s_ph, ((HEAD - 1) % 2) * 512)
                scalar.reg_mov(s_hd, ((HEAD - 1) % 2) * 64)
                scalar.reg_mov(s_tlo, HEAD)
                scalar.reg_mov(s_tst, HEAD - 2)
                scalar.reg_mov(s_thi, HEAD)
                with scalar.Fori(0, NLOOP):
                    scalar.reg_sub(s_ps, 512, s_ps)
                    scalar.reg_sub(s_ph, 512, s_ph)
                    scalar.reg_sub(s_hd, 64, s_hd)
                    scalar.reg_add(s_hd32, s_hd, 32)
                    scalar.reg_add(s_tlo, s_tlo, 1)
                    scalar.reg_add(s_tst, s_tst, 1)
                    scalar.reg_add(s_thi, s_thi, 1)
                    act_pair(
                        s_ps, s_ph, s_hd, s_hd32,
                        (lambda: scalar.wait_ge(sem_pslo, s_tlo)),
                        (lambda: scalar.wait_ge(sem_stg, s_tst)),
                        (lambda: scalar.wait_ge(sem_pshi, s_thi)),
                    )

        @block.vector
        def _(vector):
            vector.wait_ge(sem_const, 64)
            for s in range(n_slices):
                for c in range(NCH):
                    idx = 8 * s + c
                    vector.wait_ge(sem_proj, idx + 1)
                    vector.tensor_scalar_add(
                        xp_all[:, c * 4096 + s * 512: c * 4096 + (s + 1) * 512],
                        ppsum[idx % 2][:],
                        bias_sb[:, c:c + 1],
                    ).then_inc(sem_xp, 1)
            for t in range(HEAD):
                vector.wait_ge(sem_hhi, t + 1)
                if t >= 1:
                    vector.wait_ge(sem_y, 16 * t)
                vector.tensor_copy(
                    ystage[:], ap(hball, (t % 2) * 64, P_H64)
                ).then_inc(sem_stg, 1)
            if NLOOP > 0:
                v_thr = vector.alloc_register("v_thr")
                v_y = vector.alloc_register("v_y")
                v_hs = vector.alloc_register("v_hs")
                vector.reg_mov(v_thr, HEAD)
                vector.reg_mov(v_y, 16 * (HEAD - 1))
                vector.reg_mov(v_hs, ((HEAD - 1) % 2) * 64)
                with vector.Fori(0, NLOOP):
                    vector.reg_add(v_thr, v_thr, 1)
                    vector.wait_ge(sem_hhi, v_thr)
                    vector.reg_add(v_y, v_y, 16)
                    vector.wait_ge(sem_y, v_y)
                    vector.reg_sub(v_hs, 64, v_hs)
                    vector.tensor_copy(
                        ystage[:], ap(hball, v_hs, P_H64)
                    ).then_inc(sem_stg, 1)

    nc.compile()
    return nc


_PROGRAM_CACHE = {}
BUILD_KW = {"raw": True}


def _get_program(steps=T):
    key = (steps, tuple(sorted(BUILD_KW.items())))
    if key not in _PROGRAM_CACHE:
        kw = dict(BUILD_KW)
        if kw.pop("loop", False):
            builder = _build_program_loop
        elif kw.pop("raw", False):
            builder = _build_program_raw
        else:
            builder = _build_program
        _PROGRAM_CACHE[key] = builder(steps, **kw)
    return _PROGRAM_CACHE[key]


def _prep_shared(W_ih, W_hh, b_ih, b_hh):
    # lhsT layout [kappa, k*1024 + j] = W[j, k*128+kappa]
    def to_lhsT(W):
        return np.ascontiguousarray(
            W.T.reshape(8, 128, 1024).transpose(1, 0, 2).reshape(128, 8192)
        )

    wih_np = to_lhsT(np.asarray(W_ih)).astype(np.float16)
    whh_np = to_lhsT(np.asarray(W_hh)).astype(np.float16)
    bias_np = np.ascontiguousarray(
        (np.asarray(b_ih) + np.asarray(b_hh)).astype(np.float32).reshape(8, 128).T
    )
    ident_np = np.eye(128, dtype=np.float16)
    return wih_np, whh_np, bias_np, ident_np


TRACE = False
LAST_RESULT = [None]


def kernel(x, W_ih, W_hh, b_ih, b_hh, _steps=T):
    from concourse.bass_utils import run_bass_kernel_spmd

    x = np.asarray(x)
    steps = _steps
    nc = _get_program(steps)
    wih_np, whh_np, bias_np, ident_np = _prep_shared(W_ih, W_hh, b_ih, b_hh)

    in_maps = []
    for core in range(N_CORES):
        xs = x[core * BS:(core + 1) * BS]          # [8, T, I]
        # xt[kappa, k*4096 + t*8 + b] = x[b, t, k*128+kappa]
        xt_np = np.ascontiguousarray(
            xs.transpose(2, 1, 0)                   # [I, T, B]
            .reshape(8, 128, T * BS)                # [k, kappa, t*8+b]
            .transpose(1, 0, 2)                     # [kappa, k, t*8+b]
            .reshape(128, 8 * 4096)
        ).astype(np.float16)
        in_maps.append({
            "wih": wih_np, "whh": whh_np, "xt": xt_np,
            "ident": ident_np, "bias": bias_np,
        })

    res = run_bass_kernel_spmd(nc, in_maps, list(range(N_CORES)), trace=TRACE)
    LAST_RESULT[0] = res

    out = np.empty((B, T, H), dtype=np.float32)
    for core in range(N_CORES):
        yv = res.results[core]["y"]                 # [steps, 128, 64] fp16
        hb = (
            yv.reshape(steps, 128, 8, 8)
            .transpose(3, 0, 2, 1)                  # [b, t, c, kappa]
            .reshape(BS, steps, H)
            .astype(np.float32)
        )
        out[core * BS:(core + 1) * BS, :steps] = hb
    return out
